# revision 1
# baseline (speedup 1.0000x reference)
"""Multi-head attention (b=2, c=768, s=2048, 8 heads, d=96) on 8 TRN2 NeuronCores.

Sharding: batch x head-group tensor parallel. Core i handles batch i//4 and
heads {2*(i%4), 2*(i%4)+1}. Each core computes its two heads' attention plus
their contribution to the output projection; the host sums the 4 partial
outputs per batch element (the all-reduce of the sharding hint, done host-side
since the kernel returns full outputs anyway).

Per-core pipeline (all matmuls float32r ~ tf32 precision, fp32 accumulate;
measured end-to-end rel err ~6e-4):
  qT/kT = W^T @ xT          (96, 2048): x arrives already transposed as (c, s)
  S^T[j,i] = k_j . q_i      scores computed TRANSPOSED (j on partitions) so the
                            P @ V contraction needs no on-chip transposes
  P = exp(S^T)              no max-subtraction: |S| <= ~15 << fp32 overflow
  O~ = [V;1]^T @ P          ones column appended to V yields the softmax
                            denominator as PSUM row 96 of the same matmul
  o = O~[0:96] * (1/den)    denominator broadcast across partitions via a
                            K=1 ones matmul on the PE
  out += W_out_h^T @ o      accumulated over the core's 2 heads in PSUM

Schedule (TimelineSim ~128us/core; PE busy ~98us = N-column roofline of the
~229K streamed PE cycles):
  - x is DMA'd in (c-tile, 512-query-slice) chunks on the HWDGE queue while
    weights load as single batched DMAs on the SWDGE queue, so the first
    projection matmul starts ~2us in;
  - all ps_proj-pool users are emitted in exact x-arrival order (a DMA-blocked
    projection holding a pool slot would otherwise head-of-line-block ready
    work);
  - exp runs per (128, 1024) PSUM group (2 banks, 2 bufs) feeding the PV
    accumulation; qT projections for later slices are emitted after each
    attention block so the PE fills exp-wait gaps with projection work.
"""

import numpy as np

N_CORES = 8
B, C, S = 2, 768, 2048
H, D = 8, 96
CT = C // 128          # 6 c-tiles
IT = S // 512          # 4 query slices
JT = S // 128          # 16 key tiles
JG = JT // 2           # 8 exp groups of 2 key tiles

_RUNNER = None


def _split_sync_waits(nc, mybir, max_waits=1):
    """This walrus build rejects instructions carrying more than one sem wait
    (setupSyncWait: 'Too many sync wait commands'). Split excess waits onto
    same-engine NoOps inserted immediately before the instruction."""
    for bb in nc.main_func.blocks:
        insts = bb.instructions
        i = 0
        while i < len(insts):
            inst = insts[i]
            si = inst.sync_info
            if si is not None and si.on_wait and len(si.on_wait) > max_waits:
                waits = list(si.on_wait)
                keep = waits[-max_waits:]
                extra = waits[:-max_waits]
                pos = i
                while extra:
                    chunk, extra = extra[:max_waits], extra[max_waits:]
                    nop = mybir.InstNoOp(
                        name=nc.get_next_instruction_name(),
                        sync_info=mybir.SyncInfo(on_wait=chunk, on_update=[]),
                        engine=inst.engine,
                        bass_nofuse=True,
                    )
                    insts.insert(pos, nop)
                    pos += 1
                    i += 1
                si.on_wait = keep
            i += 1


DEFAULT_CFG = dict(
    dma_order="B",        # "A": wk, x(all), wq, wv, wo ; "B": wk, x0, wq, wv, x1-3, wo
    b_phase="stream_part",  # projections emitted in x-arrival order, qT 1+ as fillers
    ps_proj=2, ps_attn=2, ps_o=2,
    attn_scheme="2x2",    # "2x2": 8 groups of 2 from one pool; "3x1": 3 pools bufs=1;
                          # "mix": 7 groups of 2 (pool A bufs=2) + 2 of 1 (pool B bufs=1)
    tail_split=False, tail_pin=False,
    loop_n=1,             # benchmark mode: repeat the whole body in a HW loop
)


def _build_nc(cfg=None):
    import concourse.bass as bass
    import concourse.tile as tile
    import concourse.mybir as mybir
    from concourse.tile import add_dep_helper

    cfg = {**DEFAULT_CFG, **(cfg or {})}

    f32 = mybir.dt.float32
    f32r = mybir.dt.float32r
    EXP = mybir.ActivationFunctionType.Exp

    nc = bass.Bass(num_devices=N_CORES)
    x = nc.declare_dram_parameter("x", [C, S], f32, isOutput=False)
    wq = nc.declare_dram_parameter("wq", [C, 2 * D], f32, isOutput=False)
    wk = nc.declare_dram_parameter("wk", [C, 2 * D], f32, isOutput=False)
    wv = nc.declare_dram_parameter("wv", [C, 2 * D], f32, isOutput=False)
    wo = nc.declare_dram_parameter("wo", [2 * D, C], f32, isOutput=False)
    out = nc.declare_dram_parameter("out", [C, S], f32, isOutput=True)

    with tile.TileContext(nc) as tc:
        with (
            tc.tile_pool(name="sb_x", bufs=1) as sb_x,
            tc.tile_pool(name="sb_w", bufs=1) as sb_w,
            tc.tile_pool(name="sb_qk", bufs=1) as sb_qk,
            tc.tile_pool(name="sb_v", bufs=1) as sb_v,
            tc.tile_pool(name="sb_p", bufs=4) as sb_p,
            tc.tile_pool(name="sb_o", bufs=3) as sb_o,
            tc.tile_pool(name="sb_m", bufs=2) as sb_m,
            tc.tile_pool(name="sb_oc", bufs=6) as sb_oc,
            tc.tile_pool(name="sb_oc0", bufs=6) as sb_oc0,
            tc.tile_pool(name="ps_proj", bufs=cfg["ps_proj"], space="PSUM") as ps_proj,
            tc.tile_pool(name="ps_attn", bufs=cfg["ps_attn"], space="PSUM") as ps_attn,
            tc.tile_pool(name="ps_attn2", bufs=1, space="PSUM") as ps_attn2,
            tc.tile_pool(name="ps_attn3", bufs=1, space="PSUM") as ps_attn3,
            tc.tile_pool(name="ps_o", bufs=cfg["ps_o"], space="PSUM") as ps_o,
        ):
          import contextlib
          loop_ctx = tc.For_i(0, cfg["loop_n"], 1) if cfg["loop_n"] > 1 else contextlib.nullcontext()
          with loop_ctx:
            # fp32 constants (memset can't target f32r); DVE copies round to f32r
            czero = sb_w.tile([128, 64], f32, name="czero")
            nc.vector.memset(czero[:], 0.0)
            cone = sb_w.tile([128, 96], f32, name="cone")
            nc.vector.memset(cone[:], 1.0)
            ones1 = sb_w.tile([1, D], f32r, name="ones1")
            nc.vector.tensor_copy(ones1[:], cone[0:1, :])

            # ---- loads ----
            def load_x_slice(xt, isl, eng=None):
                eng = eng or nc.sync
                for ct in range(CT):
                    eng.dma_start(
                        xt_c[(ct, isl)][:],
                        x[ct * 128:(ct + 1) * 128, isl * 512:(isl + 1) * 512].bitcast(f32r),
                    )

            # weights load as one DMA each on the SWDGE queue (gpsimd), in
            # parallel with x streaming on the HWDGE queue (sync)
            def load_wk():
                tk = sb_w.tile([128, CT, 2 * D], f32r, name="wk")
                # c-tile 0 rides ahead on the HWDGE queue so the very first
                # projection matmul isn't gated on the whole batched load
                nc.sync.dma_start(tk[:, 0, :], wk[0:128, :].bitcast(f32r))
                nc.gpsimd.dma_start(
                    tk[:, 1:CT, :],
                    wk[128:, :].rearrange("(ct p) c -> p ct c", p=128).bitcast(f32r),
                )
                return [tk[:, ct, :] for ct in range(CT)]

            def load_wq_wv():
                tq = sb_w.tile([128, CT, 2 * D], f32r, name="wq")
                nc.gpsimd.dma_start(
                    tq[:], wq.rearrange("(ct p) c -> p ct c", p=128).bitcast(f32r)
                )
                tv = sb_w.tile([128, CT, 256], f32r, name="wv")
                nc.vector.tensor_copy(
                    tv[:, :, 2 * D:256],
                    czero[:, None, :].broadcast_to([128, CT, 64]),
                )
                nc.gpsimd.dma_start(
                    tv[:, :, 0:2 * D],
                    wv.rearrange("(ct p) c -> p ct c", p=128).bitcast(f32r),
                )
                return ([tq[:, ct, :] for ct in range(CT)],
                        [tv[:, ct, :] for ct in range(CT)])

            def load_wo():
                t = sb_w.tile([D, 2, C], f32r, name="wo")
                nc.gpsimd.dma_start(
                    t[:], wo.rearrange("(h p) c -> p h c", p=D).bitcast(f32r)
                )
                return [t[:, h, :] for h in range(2)]

            xt_c = {(ct, w): sb_x.tile([128, 512], f32r, name=f"xt{ct}_{w}")
                    for ct in range(CT) for w in range(IT)}

            class _XtView:
                """xt[ct][:, a:b] view over per-(ct, slice) tiles; slices must
                stay within one 512-wide chunk."""
                def __init__(self, ct):
                    self.ct = ct
                def __getitem__(self, key):
                    rows, cols = key
                    a, b = cols.start or 0, cols.stop
                    w, off = divmod(a, 512)
                    assert b - a <= 512 and off + (b - a) <= 512
                    return xt_c[(self.ct, w)][rows, off:off + (b - a)]

            xt = [_XtView(ct) for ct in range(CT)]
            if cfg["dma_order"] == "A":
                wk_t = load_wk()
                for isl in range(IT):
                    load_x_slice(xt, isl)
                wq_t, wv_t = load_wq_wv()
                wo_t = load_wo()
            else:
                wk_t = load_wk()
                load_x_slice(xt, 0)
                wq_t, wv_t = load_wq_wv()
                for isl in range(1, IT):
                    load_x_slice(xt, isl)
                wo_t = load_wo()

            qT = [sb_qk.tile([D, S], f32r, name=f"qT{h}") for h in range(2)]
            kT = [sb_qk.tile([D, S], f32r, name=f"kT{h}") for h in range(2)]
            v_cat = [sb_v.tile([128, JT, D + 1], f32r, name=f"v{h}") for h in range(2)]
            for h in range(2):
                nc.vector.tensor_copy(v_cat[h][:, :, D], cone[:, 0:JT])

            def proj_qk(h, isl, w_t, dst, pin_after=None):
                acc = ps_proj.tile([128, 512], f32, name="ps_proj")
                for ct in range(CT):
                    mm = nc.tensor.matmul(
                        acc[0:D, :],
                        w_t[ct][:, h * D:(h + 1) * D],
                        xt[ct][:, isl * 512:(isl + 1) * 512],
                        start=(ct == 0), stop=(ct == CT - 1),
                    )
                    if ct == 0 and pin_after is not None:
                        add_dep_helper(mm.ins, pin_after.ins, sync=True,
                                       reason="pin filler projection into block")
                nc.vector.tensor_copy(dst[:, isl * 512:(isl + 1) * 512], acc[0:D, :])

            def proj_v(jt):
                accv = ps_proj.tile([128, 512], f32, name="ps_proj")
                for ct in range(CT):
                    nc.tensor.matmul(
                        accv[:, 0:256],
                        xt[ct][:, jt * 128:(jt + 1) * 128],
                        wv_t[ct][:],
                        start=(ct == 0), stop=(ct == CT - 1),
                    )
                for h in range(2):
                    nc.vector.tensor_copy(v_cat[h][:, jt, 0:D], accv[:, h * D:(h + 1) * D])

            if cfg["b_phase"] == "stream":
                for w in range(IT):
                    proj_qk(0, w, wk_t, kT[0])
                    for jt in range(4 * w, 4 * w + 4):
                        proj_v(jt)
                    proj_qk(1, w, wk_t, kT[1])
                    proj_qk(0, w, wq_t, qT[0])
                    proj_qk(1, w, wq_t, qT[1])
            elif cfg["b_phase"] == "stream_part":
                # emit ps_proj users in exact x-slice arrival order so a
                # DMA-blocked projection never holds a slot that a ready one
                # needs (head-of-line blocking); qT slices 1+ stay as in-block
                # fillers
                for w in range(IT):
                    proj_qk(0, w, wk_t, kT[0])
                    if w == 0:
                        proj_qk(0, 0, wq_t, qT[0])
                    for jt in range(4 * w, 4 * w + 4):
                        proj_v(jt)
                    proj_qk(1, w, wk_t, kT[1])
                    if w == 0:
                        proj_qk(1, 0, wq_t, qT[1])
            else:
                for isl in range(IT):
                    proj_qk(0, isl, wk_t, kT[0])
                proj_qk(0, 0, wq_t, qT[0])

            # ---- attention + output projection ----
            scheme = cfg["attn_scheme"]
            if scheme == "2x2":
                GROUPS = [list(range(g * 2, g * 2 + 2)) for g in range(JG)]
            elif scheme == "mix":
                GROUPS = [list(range(g * 2, g * 2 + 2)) for g in range(7)] + [[14], [15]]
            elif scheme == "w332":
                GROUPS = [[0, 1, 2], [3, 4, 5], [6, 7, 8], [9, 10, 11], [12, 13], [14, 15]]
            else:  # 3x1
                GROUPS = [list(range(g * 2, g * 2 + 2)) for g in range(JG)]

            def alloc_sg(gi, width):
                if scheme == "3x1":
                    pool = (ps_attn, ps_attn2, ps_attn3)[gi % 3]
                    return pool.tile([128, width], f32, name=f"sg{gi % 3}")
                if scheme == "mix" and width == 512:
                    return ps_attn2.tile([128, 512], f32, name="sg_small")
                if scheme == "w332":
                    return ps_attn.tile([128, 1536], f32, name="ps_attn")[:, 0:width]
                return ps_attn.tile([128, 1024], f32, name="ps_attn")

            def attention_block(h, isl):
                Oacc = ps_o.tile([D + 1, 512], f32, name="ps_o")
                exp0 = None
                for gi, jts in enumerate(GROUPS):
                    if cfg["b_phase"] == "filler" and isl == 0 and h == 0:
                        for jt in jts:
                            proj_v(jt)
                    width = 512 * len(jts)
                    sg = alloc_sg(gi, width)
                    for t, jt in enumerate(jts):
                        nc.tensor.matmul(
                            sg[:, t * 512:(t + 1) * 512],
                            kT[h][:, jt * 128:(jt + 1) * 128],
                            qT[h][:, isl * 512:(isl + 1) * 512],
                            start=True, stop=True,
                        )
                    pt = sb_p.tile([128, 1536 if scheme == "w332" else 1024], f32r, name="pt")
                    e = nc.scalar.activation(pt[:, 0:width], sg[:, 0:width], EXP)
                    if exp0 is None:
                        exp0 = e
                    for t, jt in enumerate(jts):
                        nc.tensor.matmul(
                            Oacc[:],
                            v_cat[h][:, jt, :],
                            pt[:, t * 512:(t + 1) * 512],
                            start=(jt == 0), stop=(jt == JT - 1),
                        )
                return Oacc, exp0

            def normalize(Oacc):
                recip = sb_m.tile([1, 512], f32, name="recip")
                nc.vector.reciprocal(recip[:], Oacc[D:D + 1, :])
                recip_r = sb_m.tile([1, 512], f32r, name="recip_r")
                nc.vector.tensor_copy(recip_r[:], recip[:])
                bc_ps = ps_proj.tile([128, 512], f32, name="ps_proj")
                nc.tensor.matmul(bc_ps[0:D, :], ones1[:], recip_r[:], start=True, stop=True)
                bc = sb_m.tile([D, 512], f32, name="bc")
                nc.vector.tensor_copy(bc[:], bc_ps[0:D, :])
                o = sb_o.tile([D, 512], f32r, name="o_n")
                nc.vector.tensor_mul(o[:], Oacc[0:D, :], bc[:])
                return o

            filler = cfg["b_phase"] == "filler"
            part = cfg["b_phase"] == "stream_part"
            for isl in range(IT):
                last = isl == IT - 1
                O0, e0 = attention_block(0, isl)
                if filler:
                    if isl == 0:
                        for isl2 in range(IT):
                            proj_qk(1, isl2, wk_t, kT[1])
                        proj_qk(1, 0, wq_t, qT[1])
                    else:
                        proj_qk(1, isl, wq_t, qT[1])
                elif part and isl > 0:
                    proj_qk(1, isl, wq_t, qT[1])
                o0 = normalize(O0)

                oc0 = None
                if last and cfg["tail_split"]:
                    oc0 = []

                O1, e1 = attention_block(1, isl)
                if (filler or part) and not last:
                    proj_qk(0, isl + 1, wq_t, qT[0])
                if oc0 is not None:
                    for ct in range(CT):
                        po = ps_proj.tile([128, 512], f32, name="ps_proj")
                        mm = nc.tensor.matmul(
                            po[:], wo_t[0][:, ct * 128:(ct + 1) * 128], o0[:],
                            start=True, stop=True,
                        )
                        if ct == 0 and cfg["tail_pin"]:
                            add_dep_helper(mm.ins, e1.ins, sync=True,
                                           reason="pin tail h0 out-proj into h1 block")
                        t0 = sb_oc0.tile([128, 512], f32, name="oc0")
                        nc.vector.tensor_copy(t0[:], po[:])
                        oc0.append(t0)
                o1 = normalize(O1)

                for ct in range(CT):
                    if last and ct % 2 == 1:
                        po = ps_attn.tile([128, 1024], f32, name="ps_attn")[:, 0:512]
                    else:
                        po = ps_proj.tile([128, 512], f32, name="ps_proj")
                    if oc0 is not None:
                        nc.tensor.matmul(
                            po[:], wo_t[1][:, ct * 128:(ct + 1) * 128], o1[:],
                            start=True, stop=True,
                        )
                        oc = sb_oc.tile([128, 512], f32, name="oc")
                        nc.vector.tensor_add(oc[:], po[:], oc0[ct][:])
                    else:
                        for h, o in ((0, o0), (1, o1)):
                            nc.tensor.matmul(
                                po[:],
                                wo_t[h][:, ct * 128:(ct + 1) * 128],
                                o[:],
                                start=(h == 0), stop=(h == 1),
                            )
                        oc = sb_oc.tile([128, 512], f32, name="oc")
                        nc.vector.tensor_copy(oc[:], po[:])
                    nc.sync.dma_start(
                        out[ct * 128:(ct + 1) * 128, isl * 512:(isl + 1) * 512], oc[:]
                    )

    _split_sync_waits(nc, mybir)
    return nc


class _Runner:
    """Compile once, run many. Mirrors run_bass_via_pjrt's multi-core path but
    keeps the jitted executable cached across calls."""

    def __init__(self, cfg=None):
        import jax
        import concourse.mybir as mybir
        from concourse import bass2jax
        from jax.sharding import Mesh, PartitionSpec
        from jax.experimental.shard_map import shard_map

        self.jax = jax
        nc = _build_nc(cfg)
        self.nc = nc
        bass2jax.install_neuronx_cc_hook()

        in_names, out_names, out_avals = [], [], []
        for alloc in nc.m.functions[0].allocations:
            if not isinstance(alloc, mybir.MemoryLocationSet):
                continue
            name = alloc.memorylocations[0].name
            if alloc.kind == "ExternalInput":
                if nc.partition_id_tensor is None or name != nc.partition_id_tensor.name:
                    in_names.append(name)
            elif alloc.kind == "ExternalOutput":
                out_names.append(name)
                out_avals.append(
                    jax.core.ShapedArray(tuple(alloc.tensor_shape), mybir.dt.np(alloc.dtype))
                )
        self.in_names = in_names
        self.out_names = out_names
        partition_name = nc.partition_id_tensor.name if nc.partition_id_tensor else None
        all_names = tuple(in_names + out_names + ([partition_name] if partition_name else []))

        def _body(*args):
            operands = list(args)
            if partition_name is not None:
                operands.append(bass2jax.partition_id_tensor())
            outs = bass2jax._bass_exec_p.bind(
                *operands,
                out_avals=tuple(out_avals),
                in_names=all_names,
                out_names=tuple(out_names),
                lowering_input_output_aliases=(),
                sim_require_finite=True,
                sim_require_nnan=True,
                nc=nc,
            )
            return tuple(outs)

        devices = jax.devices()[:N_CORES]
        mesh = Mesh(np.asarray(devices), ("core",))
        n_all = len(in_names) + len(out_names)
        self.sharded = jax.jit(
            shard_map(
                _body,
                mesh=mesh,
                in_specs=(PartitionSpec("core"),) * n_all,
                out_specs=(PartitionSpec("core"),) * len(out_names),
                check_rep=False,
            ),
            keep_unused=True,
        )
        self.out_shapes = [tuple(a.shape) for a in out_avals]
        self.out_dtypes = [a.dtype for a in out_avals]

    def run(self, in_maps):
        concat_in = [
            np.concatenate([np.asarray(in_maps[c][n]) for c in range(N_CORES)], axis=0)
            for n in self.in_names
        ]
        concat_zero = [
            np.zeros((N_CORES * s[0], *s[1:]), d)
            for s, d in zip(self.out_shapes, self.out_dtypes)
        ]
        outs = self.sharded(*concat_in, *concat_zero)
        self.jax.block_until_ready(outs)
        return [
            {
                n: np.asarray(outs[i]).reshape(N_CORES, *self.out_shapes[i])[c]
                for i, n in enumerate(self.out_names)
            }
            for c in range(N_CORES)
        ]


def _get_runner():
    global _RUNNER
    if _RUNNER is None:
        _RUNNER = _Runner()
    return _RUNNER


def _shard_inputs(inputs, W_qkv, W_out):
    in_maps = []
    for core in range(N_CORES):
        b, g = divmod(core, 4)
        cols = slice(g * 2 * D, (g + 1) * 2 * D)
        in_maps.append({
            "x": np.ascontiguousarray(inputs[b]),
            "wq": np.ascontiguousarray(W_qkv[:, cols]),
            "wk": np.ascontiguousarray(W_qkv[:, 768:][:, cols]),
            "wv": np.ascontiguousarray(W_qkv[:, 1536:][:, cols]),
            "wo": np.ascontiguousarray(W_out[cols, :]),
        })
    return in_maps


def kernel(inputs, W_qkv, W_out):
    inputs = np.asarray(inputs, dtype=np.float32)
    W_qkv = np.asarray(W_qkv, dtype=np.float32)
    W_out = np.asarray(W_out, dtype=np.float32)
    runner = _get_runner()
    results = runner.run(_shard_inputs(inputs, W_qkv, W_out))
    out = np.zeros((B, C, S), np.float32)
    for core in range(N_CORES):
        out[core // 4] += results[core]["out"]
    return out



# revision 30
# speedup vs baseline: 1.1414x; 1.1414x over previous
"""Multi-head attention (b=2, c=768, s=2048, 8 heads, d=96) on 8 TRN2 NeuronCores.

Sharding: batch x head-group tensor parallel. Core i handles batch i//4 and
heads {2*(i%4), 2*(i%4)+1}; the host sums the 4 partial outputs per batch
element (the all-reduce of the sharding hint, done host-side since the kernel
returns full outputs anyway).

v2 schedule (vs the v1 baseline at 127.9us TimelineSim):
  - x and the QKV weights are uploaded as bf16 (host-converted): halves the
    input DMA stream and lets the v-projection run at N=192 without the f32r
    N>=256 zero-padding. Scores / PV / out-projection stay f32r.
  - output stores are bf16 (host upcasts and sums): halves store traffic so
    the final-slice store burst shrinks.
  - PE warmup: dummy matmuls on a zero tile keep the tensor engine's p-state
    ramp warm while the first x chunks stream in.
  - attention is emitted as head-interleaved rounds per query slice with a
    one-round PV lag, so each score group's exp (scalar engine) has two full
    rounds to complete; isl0's rounds start as soon as each key slice's
    projections land (exp work starts ~8us in, not ~24us).
  - each slice's normalize + out-projection is deferred into the next slice's
    rounds as PE filler so the scalar engine (the attention-phase bottleneck
    at ~66us busy) never stalls the PE.
  - tail: the last slice's output copies alternate DVE/Act and its stores
    alternate the sync (HWDGE) and gpsimd (SWDGE) queues so descriptor
    generation runs in two lanes.
"""

import numpy as np

N_CORES = 8
B, C, S = 2, 768, 2048
H, D = 8, 96
CT = C // 128          # 6 c-tiles
IT = S // 512          # 4 query slices
JT = S // 128          # 16 key tiles
JG = JT // 2           # 8 exp groups of 2 key tiles

_RUNNER = None


def _split_sync_waits(nc, mybir, max_waits=1):
    """This walrus build rejects instructions carrying more than one sem wait
    (setupSyncWait: 'Too many sync wait commands'). Split excess waits onto
    same-engine NoOps inserted immediately before the instruction."""
    for bb in nc.main_func.blocks:
        insts = bb.instructions
        i = 0
        while i < len(insts):
            inst = insts[i]
            si = inst.sync_info
            if si is not None and si.on_wait and len(si.on_wait) > max_waits:
                waits = list(si.on_wait)
                keep = waits[-max_waits:]
                extra = waits[:-max_waits]
                pos = i
                while extra:
                    chunk, extra = extra[:max_waits], extra[max_waits:]
                    nop = mybir.InstNoOp(
                        name=nc.get_next_instruction_name(),
                        sync_info=mybir.SyncInfo(on_wait=chunk, on_update=[]),
                        engine=inst.engine,
                        bass_nofuse=True,
                    )
                    insts.insert(pos, nop)
                    pos += 1
                    i += 1
                si.on_wait = keep
            i += 1


DEFAULT_CFG = dict(
    warm0=22,            # warmup dummy matmuls before the first projection
    warm_trickle=1,      # dummies interleaved after each slice-0 c-tile matmul
    gate_wqv=4,          # x-s0 chunk whose DMA gates the wq/wv SWDGE loads
    gate_wo=1,           # x slice whose last chunk gates the wo SWDGE load
    tail_act_copies=3,   # of the 6 tail out copies, how many go on Act
    tail_swdge=2,        # of the 6 tail stores, how many go on the gpsimd queue
    loop_n=1,
)


def _build_nc(cfg=None):
    import concourse.bass as bass
    import concourse.tile as tile
    import concourse.mybir as mybir
    from concourse.tile import add_dep_helper

    cfg = {**DEFAULT_CFG, **(cfg or {})}

    f32 = mybir.dt.float32
    f32r = mybir.dt.float32r
    bf16 = mybir.dt.bfloat16
    EXP = mybir.ActivationFunctionType.Exp
    COPY = mybir.ActivationFunctionType.Copy

    # weights arrive host-packed partition-major so every DMA descriptor is a
    # full 2KB+ contiguous row (small descriptors pay a 2x DMA penalty):
    #   wq/wk/wv: [128, ct*192+j] = W[ct*128+p, j]   (bf16)
    #   wo:       [96, h*768+c]  = W_out[h*96+p, c]  (f32)
    nc = bass.Bass(num_devices=N_CORES)
    x = nc.declare_dram_parameter("x", [C, S], bf16, isOutput=False)
    wq = nc.declare_dram_parameter("wq", [128, CT * 2 * D], bf16, isOutput=False)
    wk = nc.declare_dram_parameter("wk", [128, CT * 2 * D], bf16, isOutput=False)
    wv = nc.declare_dram_parameter("wv", [128, CT * 2 * D], bf16, isOutput=False)
    wo = nc.declare_dram_parameter("wo", [D, 2 * C], f32, isOutput=False)
    out = nc.declare_dram_parameter("out", [C, S], bf16, isOutput=True)

    with tile.TileContext(nc) as tc:
        with (
            tc.tile_pool(name="sb_x", bufs=1) as sb_x,
            tc.tile_pool(name="sb_w", bufs=1) as sb_w,
            tc.tile_pool(name="sb_qk", bufs=1) as sb_qk,
            tc.tile_pool(name="sb_v", bufs=1) as sb_v,
            tc.tile_pool(name="sb_p", bufs=4) as sb_p,
            tc.tile_pool(name="sb_o", bufs=3) as sb_o,
            tc.tile_pool(name="sb_m", bufs=4) as sb_m,
            tc.tile_pool(name="sb_oc", bufs=8) as sb_oc,
            tc.tile_pool(name="ps_proj", bufs=2, space="PSUM") as ps_proj,
            tc.tile_pool(name="ps_attn", bufs=2, space="PSUM") as ps_attn,
            tc.tile_pool(name="ps_o", bufs=2, space="PSUM") as ps_o,
        ):
          import contextlib
          loop_ctx = tc.For_i(0, cfg["loop_n"], 1) if cfg["loop_n"] > 1 else contextlib.nullcontext()
          with loop_ctx:
            # fp32 constants (memset can't target f32r); DVE copies round.
            # zr is a single partition row: the warmup matmul contracts K=1.
            zf = sb_w.tile([1, 64], f32, name="zf")
            nc.vector.memset(zf[:], 0.0)
            zr = sb_w.tile([1, 64], f32r, name="zr")
            nc.vector.tensor_copy(zr[:], zf[:])
            cone = sb_w.tile([128, 96], f32, name="cone")
            nc.vector.memset(cone[:], 1.0)
            ones1 = sb_w.tile([1, D], f32r, name="ones1")
            nc.vector.tensor_copy(ones1[:], cone[0:1, :])

            def dummy_mm(n=1):
                """PE p-state warmers: small f32r matmul on the zero tile."""
                for _ in range(n):
                    dps = ps_proj.tile([128, 512], f32, name="ps_proj")
                    nc.tensor.matmul(dps[0:64, 0:64], zr[:], zr[:],
                                     start=True, stop=True)

            # ---- input DMAs ----
            # slice 0 as per-(ct,512) chunks for fast availability; the rest
            # as two (128, 768) DMAs per c-tile (fewer HWDGE generations than
            # per-slice chunks, smoother arrival than one wide DMA)
            xt_s0 = {ct: sb_x.tile([128, 512], bf16, name=f"xt{ct}_0")
                     for ct in range(CT)}
            xt_rest = {ct: sb_x.tile([128, 3 * 512], bf16, name=f"xt{ct}_r")
                       for ct in range(CT)}
            x_dmas = {}

            def load_x_slice0():
                for ct in range(CT):
                    x_dmas[(ct, 0)] = nc.sync.dma_start(
                        xt_s0[ct][:],
                        x[ct * 128:(ct + 1) * 128, 0:512],
                    )

            def load_x_rest(ct, half):
                a, b = (0, 768) if half == 0 else (768, 1536)
                d = nc.sync.dma_start(
                    xt_rest[ct][:, a:b],
                    x[ct * 128:(ct + 1) * 128, 512 + a:512 + b],
                )
                # half 0 covers slice 1 and half of slice 2; half 1 the rest
                if half == 0:
                    x_dmas[(ct, 1)] = d
                else:
                    x_dmas[(ct, 2)] = d
                    x_dmas[(ct, 3)] = d

            class _XtView:
                def __init__(self, ct):
                    self.ct = ct
                def __getitem__(self, key):
                    rows, cols = key
                    a, b = cols.start or 0, cols.stop
                    assert b - a <= 512
                    if b <= 512:
                        return xt_s0[self.ct][rows, a:b]
                    assert a >= 512
                    return xt_rest[self.ct][rows, a - 512:b - 512]

            xt = [_XtView(ct) for ct in range(CT)]

            # weights: wk on SWDGE immediately (its transfer slots between the
            # first x chunks); wq/wv gated on a later x-s0 chunk so slice 0
            # completes first; wo gated on the x-rest loads.
            tk = sb_w.tile([128, CT * 2 * D], bf16, name="wk")
            nc.gpsimd.dma_start(tk[:], wk[:])
            wk_t = [tk[:, ct * 2 * D:(ct + 1) * 2 * D] for ct in range(CT)]

            load_x_slice0()

            # wv before wq: the per-slice emission consumes v before q
            tv = sb_w.tile([128, CT * 2 * D], bf16, name="wv")
            d_wv = nc.gpsimd.dma_start(tv[:], wv[:])
            tq = sb_w.tile([128, CT * 2 * D], bf16, name="wq")
            d_wq = nc.gpsimd.dma_start(tq[:], wq[:])
            gate = x_dmas[(cfg["gate_wqv"], 0)]
            add_dep_helper(d_wq.ins, gate.ins, sync=True, reason="wqv after x s0")
            add_dep_helper(d_wv.ins, gate.ins, sync=True, reason="wqv after x s0")
            wq_t = [tq[:, ct * 2 * D:(ct + 1) * 2 * D] for ct in range(CT)]
            wv_t = [tv[:, ct * 2 * D:(ct + 1) * 2 * D] for ct in range(CT)]

            for ct in range(CT):
                load_x_rest(ct, 0)
            for ct in range(CT):
                load_x_rest(ct, 1)

            two = sb_w.tile([D, 2 * C], f32r, name="wo")
            d_wo = nc.gpsimd.dma_start(two[:], wo[:].bitcast(f32r))
            add_dep_helper(d_wo.ins, x_dmas[(CT - 1, cfg["gate_wo"])].ins,
                           sync=True, reason="wo after x")
            wo_t = [two[:, h * C:(h + 1) * C] for h in range(2)]

            # ---- persistent compute tiles ----
            qT = [sb_qk.tile([D, S], f32r, name=f"qT{h}") for h in range(2)]
            kT = [sb_qk.tile([D, S], f32r, name=f"kT{h}") for h in range(2)]
            v_cat = [sb_v.tile([128, JT, D + 1], f32r, name=f"v{h}") for h in range(2)]
            for h in range(2):
                nc.vector.tensor_copy(v_cat[h][:, :, D], cone[:, 0:JT])

            def proj_qk(h, isl, w_t, dst, trickle=0):
                acc = ps_proj.tile([128, 512], f32, name="ps_proj")
                for ct in range(CT):
                    nc.tensor.matmul(
                        acc[0:D, :],
                        w_t[ct][:, h * D:(h + 1) * D],
                        xt[ct][:, isl * 512:(isl + 1) * 512],
                        start=(ct == 0), stop=(ct == CT - 1),
                    )
                    if trickle:
                        dummy_mm(trickle)
                nc.vector.tensor_copy(dst[:, isl * 512:(isl + 1) * 512], acc[0:D, :])

            def proj_v(jt):
                accv = ps_proj.tile([128, 512], f32, name="ps_proj")
                for ct in range(CT):
                    nc.tensor.matmul(
                        accv[:, 0:2 * D],
                        xt[ct][:, jt * 128:(jt + 1) * 128],
                        wv_t[ct][:],
                        start=(ct == 0), stop=(ct == CT - 1),
                    )
                for h in range(2):
                    nc.vector.tensor_copy(v_cat[h][:, jt, 0:D], accv[:, h * D:(h + 1) * D])

            # ---- attention machinery ----
            # score groups: (h, isl, g) covers key tiles jt in {2g, 2g+1}
            sg_tiles = {}
            exp_tiles = {}

            def emit_scores(h, isl, g):
                sg = ps_attn.tile([128, 1024], f32, name="ps_attn")
                for t, jt in enumerate((2 * g, 2 * g + 1)):
                    nc.tensor.matmul(
                        sg[:, t * 512:(t + 1) * 512],
                        kT[h][:, jt * 128:(jt + 1) * 128],
                        qT[h][:, isl * 512:(isl + 1) * 512],
                        start=True, stop=True,
                    )
                pt = sb_p.tile([128, 1024], f32r, name="pt")
                nc.scalar.activation(pt[:], sg[:], EXP)
                sg_tiles[(h, isl, g)] = sg
                exp_tiles[(h, isl, g)] = pt

            oacc = {}

            def emit_pv(h, isl, g):
                if g == 0:
                    oacc[(h, isl)] = ps_o.tile([D + 1, 512], f32, name="ps_o")
                pt = exp_tiles.pop((h, isl, g))
                del sg_tiles[(h, isl, g)]
                Oacc = oacc[(h, isl)]
                for t, jt in enumerate((2 * g, 2 * g + 1)):
                    nc.tensor.matmul(
                        Oacc[:],
                        v_cat[h][:, jt, :],
                        pt[:, t * 512:(t + 1) * 512],
                        start=(jt == 0), stop=(jt == JT - 1),
                    )

            def emit_recip(h, isl):
                Oacc = oacc[(h, isl)]
                recip_r = sb_m.tile([1, 512], f32r, name="recip_r")
                with nc.allow_low_precision("softmax denominator reciprocal"):
                    nc.vector.reciprocal(recip_r[:], Oacc[D:D + 1, :])
                return recip_r

            def emit_bc(recip_r):
                bc_ps = ps_proj.tile([128, 512], f32, name="ps_proj")
                nc.tensor.matmul(bc_ps[0:D, :], ones1[:], recip_r[:],
                                 start=True, stop=True)
                return bc_ps

            def emit_mul(h, isl, bc_ps, copy_eng=None):
                # the DVE can read only one PSUM operand: copy the
                # unnormalized Oacc to SBUF (in parallel with the reciprocal /
                # bc broadcast), then multiply SBUF x PSUM. Also frees the
                # Oacc bank earlier.
                Oacc = oacc.pop((h, isl))
                ou = sb_m.tile([D, 512], f32, name="ou")
                if copy_eng is nc.scalar:
                    nc.scalar.activation(ou[:], Oacc[0:D, :], COPY)
                else:
                    nc.vector.tensor_copy(ou[:], Oacc[0:D, :])
                o = sb_o.tile([D, 512], f32r, name="o_n")
                nc.vector.tensor_mul(o[:], ou[:], bc_ps[0:D, :])
                return o

            def emit_outproj_ct(isl, ct, o0, o1, tail_i=None):
                po = ps_proj.tile([128, 512], f32, name="ps_proj")
                for h, o in ((0, o0), (1, o1)):
                    nc.tensor.matmul(
                        po[:],
                        wo_t[h][:, ct * 128:(ct + 1) * 128],
                        o[:],
                        start=(h == 0), stop=(h == 1),
                    )
                oc = sb_oc.tile([128, 512], bf16, name="oc")
                if tail_i is not None and tail_i < cfg["tail_act_copies"]:
                    nc.scalar.activation(oc[:], po[:], COPY)
                else:
                    nc.vector.tensor_copy(oc[:], po[:])
                dst = out[ct * 128:(ct + 1) * 128, isl * 512:(isl + 1) * 512]
                if tail_i is not None and tail_i < cfg["tail_swdge"]:
                    nc.gpsimd.dma_start(dst, oc[:])
                else:
                    nc.sync.dma_start(dst, oc[:])

            # ---- phase 1: slice-pipelined projections + isl0 attention ----
            for s in range(IT):
                trickle = cfg["warm_trickle"] if s == 0 else 0
                if s == 0:
                    dummy_mm(cfg["warm0"])
                proj_qk(0, s, wk_t, kT[0], trickle=trickle)
                proj_qk(1, s, wk_t, kT[1], trickle=trickle)
                for jt in range(4 * s, 4 * s + 4):
                    proj_v(jt)
                proj_qk(0, s, wq_t, qT[0])
                proj_qk(1, s, wq_t, qT[1])
                # isl0 rounds for the groups this slice's keys unlock
                for g in range(2 * s, 2 * s + 2):
                    emit_scores(0, 0, g)
                    emit_scores(1, 0, g)
                    if g > 0:
                        emit_pv(0, 0, g - 1)
                        emit_pv(1, 0, g - 1)
            emit_pv(0, 0, JG - 1)
            emit_pv(1, 0, JG - 1)

            # ---- phase 2: isl 1..3 rounds with previous isl's normalize +
            # out-projection + stores as fillers ----
            def norm_outproj_filler(isl):
                """Generator yielding PE filler steps for isl's tail work."""
                r0 = emit_recip(0, isl)
                r1 = emit_recip(1, isl)
                yield
                bc0 = emit_bc(r0)
                yield
                bc1 = emit_bc(r1)
                o0 = emit_mul(0, isl, bc0)
                o1 = emit_mul(1, isl, bc1)
                yield
                for ct in range(CT):
                    emit_outproj_ct(isl, ct, o0, o1)
                    yield

            for isl in range(1, IT):
                filler = norm_outproj_filler(isl - 1)
                last = isl == IT - 1
                # on the last slice h1 leads: its final exp then clears the
                # scalar engine one slot earlier, letting the h1 normalize
                # chain start while h0's last exp still runs
                ha, hb = (1, 0) if last else (0, 1)
                for g in range(JG):
                    emit_scores(ha, isl, g)
                    emit_scores(hb, isl, g)
                    if g > 0:
                        emit_pv(ha, isl, g - 1)
                        emit_pv(hb, isl, g - 1)
                    next(filler, None)
                    if g in (2, 4, 6):
                        next(filler, None)
                if not last:
                    emit_pv(ha, isl, JG - 1)
                    emit_pv(hb, isl, JG - 1)
                for _ in filler:
                    pass

            # ---- tail: isl3 normalize + out-projection + stores ----
            # h0's half of the out-projection starts as soon as o0 is ready
            # (po tiles: 2 from ps_proj + 4 carved from the now-free ps_attn
            # tiles); h1 accumulates into them once o1 lands. Copies alternate
            # DVE/Act per chunk; the earliest stores ride the SWDGE lane.
            isl = IT - 1
            po = [None] * CT

            def mm_out(h, ct, o, stop):
                nc.tensor.matmul(
                    po[ct][:], wo_t[h][:, ct * 128:(ct + 1) * 128], o[:],
                    start=(h == 0), stop=stop,
                )

            # tail pipeline, h1 leading: h1's PV finishes first, so its whole
            # normalize chain runs while h0's last exp + PV complete; h1's
            # out-proj matmuls open the psum accumulation, h0's close it with
            # copy + store chasing each closing matmul.
            emit_pv(1, isl, JG - 1)
            r1 = emit_recip(1, isl)
            bc1 = emit_bc(r1)
            emit_pv(0, isl, JG - 1)
            o1 = emit_mul(1, isl, bc1, copy_eng=nc.scalar)
            r0 = emit_recip(0, isl)
            CT_ORDER = (2, 3, 4, 5, 0, 1)
            for ct in CT_ORDER[:4]:
                if ct % 2 == 0:
                    big = ps_attn.tile([128, 1024], f32, name="ps_attn")
                    po[ct] = big[:, 0:512]
                else:
                    po[ct] = big[:, 512:1024]
                nc.tensor.matmul(
                    po[ct][:], wo_t[1][:, ct * 128:(ct + 1) * 128], o1[:],
                    start=True, stop=False,
                )
            bc0 = emit_bc(r0)
            o0 = emit_mul(0, isl, bc0, copy_eng=nc.scalar)
            for ct in CT_ORDER[4:]:
                po[ct] = ps_o.tile([128, 512], f32, name="ps_o")
                nc.tensor.matmul(
                    po[ct][:], wo_t[1][:, ct * 128:(ct + 1) * 128], o1[:],
                    start=True, stop=False,
                )
            for i, ct in enumerate(CT_ORDER):
                nc.tensor.matmul(
                    po[ct][:], wo_t[0][:, ct * 128:(ct + 1) * 128], o0[:],
                    start=False, stop=True,
                )
                oc = sb_oc.tile([128, 512], bf16, name="oc")
                if i % 2 == 0:
                    nc.scalar.activation(oc[:], po[ct][:], COPY)
                else:
                    nc.vector.tensor_copy(oc[:], po[ct][:])
                dst = out[ct * 128:(ct + 1) * 128, isl * 512:(isl + 1) * 512]
                if i < cfg["tail_swdge"]:
                    nc.gpsimd.dma_start(dst, oc[:])
                else:
                    nc.sync.dma_start(dst, oc[:])

    _split_sync_waits(nc, mybir)
    return nc


class _Runner:
    """Compile once, run many. Mirrors run_bass_via_pjrt's multi-core path but
    keeps the jitted executable cached across calls."""

    def __init__(self, cfg=None):
        import jax
        import concourse.mybir as mybir
        from concourse import bass2jax
        from jax.sharding import Mesh, PartitionSpec
        from jax.experimental.shard_map import shard_map

        self.jax = jax
        nc = _build_nc(cfg)
        self.nc = nc
        bass2jax.install_neuronx_cc_hook()

        in_names, out_names, out_avals = [], [], []
        for alloc in nc.m.functions[0].allocations:
            if not isinstance(alloc, mybir.MemoryLocationSet):
                continue
            name = alloc.memorylocations[0].name
            if alloc.kind == "ExternalInput":
                if nc.partition_id_tensor is None or name != nc.partition_id_tensor.name:
                    in_names.append(name)
            elif alloc.kind == "ExternalOutput":
                out_names.append(name)
                out_avals.append(
                    jax.core.ShapedArray(tuple(alloc.tensor_shape), mybir.dt.np(alloc.dtype))
                )
        self.in_names = in_names
        self.out_names = out_names
        partition_name = nc.partition_id_tensor.name if nc.partition_id_tensor else None
        all_names = tuple(in_names + out_names + ([partition_name] if partition_name else []))

        def _body(*args):
            operands = list(args)
            if partition_name is not None:
                operands.append(bass2jax.partition_id_tensor())
            outs = bass2jax._bass_exec_p.bind(
                *operands,
                out_avals=tuple(out_avals),
                in_names=all_names,
                out_names=tuple(out_names),
                lowering_input_output_aliases=(),
                sim_require_finite=True,
                sim_require_nnan=True,
                nc=nc,
            )
            return tuple(outs)

        devices = jax.devices()[:N_CORES]
        mesh = Mesh(np.asarray(devices), ("core",))
        n_all = len(in_names) + len(out_names)
        self.sharded = jax.jit(
            shard_map(
                _body,
                mesh=mesh,
                in_specs=(PartitionSpec("core"),) * n_all,
                out_specs=(PartitionSpec("core"),) * len(out_names),
                check_rep=False,
            ),
            keep_unused=True,
        )
        self.out_shapes = [tuple(a.shape) for a in out_avals]
        self.out_dtypes = [a.dtype for a in out_avals]

    def run(self, in_maps):
        concat_in = [
            np.concatenate([np.asarray(in_maps[c][n]) for c in range(N_CORES)], axis=0)
            for n in self.in_names
        ]
        concat_zero = [
            np.zeros((N_CORES * s[0], *s[1:]), d)
            for s, d in zip(self.out_shapes, self.out_dtypes)
        ]
        outs = self.sharded(*concat_in, *concat_zero)
        self.jax.block_until_ready(outs)
        return [
            {
                n: np.asarray(outs[i]).reshape(N_CORES, *self.out_shapes[i])[c]
                for i, n in enumerate(self.out_names)
            }
            for c in range(N_CORES)
        ]


def _get_runner():
    global _RUNNER
    if _RUNNER is None:
        _RUNNER = _Runner()
    return _RUNNER


def _pack_w(w):
    """(768, 192) -> (128, 6*192) partition-major: out[p, ct*192+j] = w[ct*128+p, j]."""
    return np.ascontiguousarray(
        w.reshape(CT, 128, 2 * D).transpose(1, 0, 2).reshape(128, CT * 2 * D)
    )


def _shard_inputs(inputs, W_qkv, W_out):
    import ml_dtypes

    bf16 = ml_dtypes.bfloat16
    in_maps = []
    for core in range(N_CORES):
        b, g = divmod(core, 4)
        cols = slice(g * 2 * D, (g + 1) * 2 * D)
        wo = W_out[cols, :]  # (192, 768)
        wo_packed = np.ascontiguousarray(
            wo.reshape(2, D, C).transpose(1, 0, 2).reshape(D, 2 * C)
        )
        in_maps.append({
            "x": np.ascontiguousarray(inputs[b]).astype(bf16),
            "wq": _pack_w(W_qkv[:, cols]).astype(bf16),
            "wk": _pack_w(W_qkv[:, 768:][:, cols]).astype(bf16),
            "wv": _pack_w(W_qkv[:, 1536:][:, cols]).astype(bf16),
            "wo": wo_packed,
        })
    return in_maps


def kernel(inputs, W_qkv, W_out):
    inputs = np.asarray(inputs, dtype=np.float32)
    W_qkv = np.asarray(W_qkv, dtype=np.float32)
    W_out = np.asarray(W_out, dtype=np.float32)
    runner = _get_runner()
    results = runner.run(_shard_inputs(inputs, W_qkv, W_out))
    out = np.zeros((B, C, S), np.float32)
    for core in range(N_CORES):
        out[core // 4] += results[core]["out"].astype(np.float32)
    return out


# revision 53
# speedup vs baseline: 1.1552x; 1.0121x over previous
"""Multi-head attention (b=2, c=768, s=2048, 8 heads, d=96) on 8 TRN2 NeuronCores.

Sharding: batch x head-group tensor parallel. Core i handles batch i//4 and
heads {2*(i%4), 2*(i%4)+1}; the host sums the 4 partial outputs per batch
element (the all-reduce of the sharding hint, done host-side since the kernel
returns full outputs anyway).

v2 schedule (111.0us TimelineSim vs the v1 baseline's 127.9us; measured HW
rel err ~9e-3 against the f32 reference, tolerance 2e-2):
  - x and the QKV weights are uploaded as bf16 (host-converted, host-packed
    partition-major so every DMA descriptor is a full 2KB+ row): halves the
    input DMA stream and lets the v-projection run at N=192 without the f32r
    N>=256 zero-padding. Scores / PV / out-projection stay f32r.
  - output stores are bf16 (host upcasts and sums the per-core partials):
    halves store traffic so the final-slice store burst shrinks.
  - PE warmup: dummy matmuls on a zero tile keep the tensor engine's p-state
    ramp warm while the first x chunks stream in; slice 0 loads as 512-wide
    chunks, the rest as 768-wide chunks (HWDGE generation, 625ns per DMA, is
    the input-stream limiter once transfers are bf16).
  - attention is emitted as a decoupled lead stream (scores + exp, as early
    as each key slice's projections land — exp starts ~8us in) and a trail
    stream (PV + normalize + out-projection + stores) that lags `lag` score
    groups behind, buffered in the deep sb_p pt pool. The trail doubles as
    the PE filler that paces the lead to the scalar engine's exp throughput
    (the attention-phase bottleneck at ~66us busy), so the scalar engine
    finishes its exp queue mid-kernel instead of gating the tail.
  - normalize: the softmax denominator row (PV's ones-column) is
    reciprocal'd on DVE, broadcast across partitions via a K=1 matmul, and
    multiplied against an SBUF copy of the unnormalized Oacc (the DVE can
    read only one PSUM operand per instruction).
  - tail: h0's normalize + opening out-proj matmuls overlap h1's final
    exp/PV; po tiles borrow the freed ps_attn banks; copies alternate
    Act/DVE per chunk and the first stores ride the gpsimd SWDGE lane so
    descriptor generation runs in two lanes.
"""

import numpy as np

N_CORES = 8
B, C, S = 2, 768, 2048
H, D = 8, 96
CT = C // 128          # 6 c-tiles
IT = S // 512          # 4 query slices
JT = S // 128          # 16 key tiles
JG = JT // 2           # 8 exp groups of 2 key tiles

_RUNNER = None


def _split_sync_waits(nc, mybir, max_waits=1):
    """This walrus build rejects instructions carrying more than one sem wait
    (setupSyncWait: 'Too many sync wait commands'). Split excess waits onto
    same-engine NoOps inserted immediately before the instruction."""
    for bb in nc.main_func.blocks:
        insts = bb.instructions
        i = 0
        while i < len(insts):
            inst = insts[i]
            si = inst.sync_info
            if si is not None and si.on_wait and len(si.on_wait) > max_waits:
                waits = list(si.on_wait)
                keep = waits[-max_waits:]
                extra = waits[:-max_waits]
                pos = i
                while extra:
                    chunk, extra = extra[:max_waits], extra[max_waits:]
                    nop = mybir.InstNoOp(
                        name=nc.get_next_instruction_name(),
                        sync_info=mybir.SyncInfo(on_wait=chunk, on_update=[]),
                        engine=inst.engine,
                        bass_nofuse=True,
                    )
                    insts.insert(pos, nop)
                    pos += 1
                    i += 1
                si.on_wait = keep
            i += 1


DEFAULT_CFG = dict(
    warm0=22,            # warmup dummy matmuls before the first projection
    warm_trickle=1,      # dummies interleaved after each slice-0 c-tile matmul
    lag=6,               # score groups the PV/outproj trail runs behind
    taper_mult=1,        # how aggressively the trail drains near the end
    gate_wqv=2,          # x-s0 chunk whose DMA gates the wq/wv SWDGE loads
    gate_wo=1,           # x slice whose last chunk gates the wo SWDGE load
    tail_act_copies=3,   # of the 6 tail out copies, how many go on Act
    tail_swdge=2,        # of the 6 tail stores, how many go on the gpsimd queue
    loop_n=1,
)


def _build_nc(cfg=None):
    import concourse.bass as bass
    import concourse.tile as tile
    import concourse.mybir as mybir
    from concourse.tile import add_dep_helper

    cfg = {**DEFAULT_CFG, **(cfg or {})}

    f32 = mybir.dt.float32
    f32r = mybir.dt.float32r
    bf16 = mybir.dt.bfloat16
    EXP = mybir.ActivationFunctionType.Exp
    COPY = mybir.ActivationFunctionType.Copy

    # weights arrive host-packed partition-major so every DMA descriptor is a
    # full 2KB+ contiguous row (small descriptors pay a 2x DMA penalty):
    #   wq/wk/wv: [128, ct*192+j] = W[ct*128+p, j]   (bf16)
    #   wo:       [96, h*768+c]  = W_out[h*96+p, c]  (f32)
    nc = bass.Bass(num_devices=N_CORES)
    x = nc.declare_dram_parameter("x", [C, S], bf16, isOutput=False)
    wq = nc.declare_dram_parameter("wq", [128, CT * 2 * D], bf16, isOutput=False)
    wk = nc.declare_dram_parameter("wk", [128, CT * 2 * D], bf16, isOutput=False)
    wv = nc.declare_dram_parameter("wv", [128, CT * 2 * D], bf16, isOutput=False)
    wo = nc.declare_dram_parameter("wo", [D, 2 * C], f32, isOutput=False)
    out = nc.declare_dram_parameter("out", [C, S], bf16, isOutput=True)

    with tile.TileContext(nc) as tc:
        with (
            tc.tile_pool(name="sb_x", bufs=1) as sb_x,
            tc.tile_pool(name="sb_w", bufs=1) as sb_w,
            tc.tile_pool(name="sb_qk", bufs=1) as sb_qk,
            tc.tile_pool(name="sb_v", bufs=1) as sb_v,
            tc.tile_pool(name="sb_p", bufs=cfg["lag"] + 2) as sb_p,
            tc.tile_pool(name="sb_o", bufs=3) as sb_o,
            tc.tile_pool(name="sb_m", bufs=6) as sb_m,
            tc.tile_pool(name="sb_oc", bufs=8) as sb_oc,
            tc.tile_pool(name="ps_proj", bufs=2, space="PSUM") as ps_proj,
            tc.tile_pool(name="ps_attn", bufs=2, space="PSUM") as ps_attn,
            tc.tile_pool(name="ps_o", bufs=2, space="PSUM") as ps_o,
        ):
          import contextlib
          loop_ctx = tc.For_i(0, cfg["loop_n"], 1) if cfg["loop_n"] > 1 else contextlib.nullcontext()
          with loop_ctx:
            # bf16 memset works directly (f32r doesn't), saving the
            # f32->f32r copy on the warmup critical path.
            # zr is a single partition row: the warmup matmul contracts K=1.
            zr = sb_w.tile([1, 64], bf16, name="zr")
            nc.vector.memset(zr[:], 0.0)
            cone = sb_w.tile([128, 96], f32, name="cone")
            nc.vector.memset(cone[:], 1.0)
            ones1 = sb_w.tile([1, D], f32r, name="ones1")
            nc.vector.tensor_copy(ones1[:], cone[0:1, :])

            def dummy_mm(n=1):
                """PE p-state warmers: small f32r matmul on the zero tile."""
                for _ in range(n):
                    dps = ps_proj.tile([128, 512], f32, name="ps_proj")
                    nc.tensor.matmul(dps[0:64, 0:64], zr[:], zr[:],
                                     start=True, stop=True)

            # ---- input DMAs ----
            # slice 0 as per-(ct,512) chunks for fast availability; the rest
            # as two (128, 768) DMAs per c-tile (fewer HWDGE generations than
            # per-slice chunks, smoother arrival than one wide DMA)
            xt_s0 = {ct: sb_x.tile([128, 512], bf16, name=f"xt{ct}_0")
                     for ct in range(CT)}
            xt_rest = {ct: sb_x.tile([128, 3 * 512], bf16, name=f"xt{ct}_r")
                       for ct in range(CT)}
            x_dmas = {}

            def load_x_slice0():
                for ct in range(CT):
                    x_dmas[(ct, 0)] = nc.sync.dma_start(
                        xt_s0[ct][:],
                        x[ct * 128:(ct + 1) * 128, 0:512],
                    )

            def load_x_rest(ct, half):
                a, b = (0, 768) if half == 0 else (768, 1536)
                d = nc.sync.dma_start(
                    xt_rest[ct][:, a:b],
                    x[ct * 128:(ct + 1) * 128, 512 + a:512 + b],
                )
                # half 0 covers slice 1 and half of slice 2; half 1 the rest
                if half == 0:
                    x_dmas[(ct, 1)] = d
                else:
                    x_dmas[(ct, 2)] = d
                    x_dmas[(ct, 3)] = d

            class _XtView:
                def __init__(self, ct):
                    self.ct = ct
                def __getitem__(self, key):
                    rows, cols = key
                    a, b = cols.start or 0, cols.stop
                    assert b - a <= 512
                    if b <= 512:
                        return xt_s0[self.ct][rows, a:b]
                    assert a >= 512
                    return xt_rest[self.ct][rows, a - 512:b - 512]

            xt = [_XtView(ct) for ct in range(CT)]

            # weights: wk on SWDGE immediately (its transfer slots between the
            # first x chunks); wq/wv gated on a later x-s0 chunk so slice 0
            # completes first; wo gated on the x-rest loads.
            tk = sb_w.tile([128, CT * 2 * D], bf16, name="wk")
            nc.gpsimd.dma_start(tk[:], wk[:])
            wk_t = [tk[:, ct * 2 * D:(ct + 1) * 2 * D] for ct in range(CT)]

            load_x_slice0()

            # wv before wq: the per-slice emission consumes v before q
            tv = sb_w.tile([128, CT * 2 * D], bf16, name="wv")
            d_wv = nc.gpsimd.dma_start(tv[:], wv[:])
            tq = sb_w.tile([128, CT * 2 * D], bf16, name="wq")
            d_wq = nc.gpsimd.dma_start(tq[:], wq[:])
            gate = x_dmas[(cfg["gate_wqv"], 0)]
            add_dep_helper(d_wq.ins, gate.ins, sync=True, reason="wqv after x s0")
            add_dep_helper(d_wv.ins, gate.ins, sync=True, reason="wqv after x s0")
            wq_t = [tq[:, ct * 2 * D:(ct + 1) * 2 * D] for ct in range(CT)]
            wv_t = [tv[:, ct * 2 * D:(ct + 1) * 2 * D] for ct in range(CT)]

            for ct in range(CT):
                load_x_rest(ct, 0)
            for ct in range(CT):
                load_x_rest(ct, 1)

            two = sb_w.tile([D, 2 * C], f32r, name="wo")
            d_wo = nc.gpsimd.dma_start(two[:], wo[:].bitcast(f32r))
            add_dep_helper(d_wo.ins, x_dmas[(CT - 1, cfg["gate_wo"])].ins,
                           sync=True, reason="wo after x")
            wo_t = [two[:, h * C:(h + 1) * C] for h in range(2)]

            # ---- persistent compute tiles ----
            qT = [sb_qk.tile([D, S], f32r, name=f"qT{h}") for h in range(2)]
            kT = [sb_qk.tile([D, S], f32r, name=f"kT{h}") for h in range(2)]
            v_cat = [sb_v.tile([128, JT, D + 1], f32r, name=f"v{h}") for h in range(2)]
            for h in range(2):
                nc.vector.tensor_copy(v_cat[h][:, :, D], cone[:, 0:JT])

            def proj_qk(h, isl, w_t, dst, trickle=0):
                acc = ps_proj.tile([128, 512], f32, name="ps_proj")
                for ct in range(CT):
                    nc.tensor.matmul(
                        acc[0:D, :],
                        w_t[ct][:, h * D:(h + 1) * D],
                        xt[ct][:, isl * 512:(isl + 1) * 512],
                        start=(ct == 0), stop=(ct == CT - 1),
                    )
                    if trickle:
                        dummy_mm(trickle)
                nc.vector.tensor_copy(dst[:, isl * 512:(isl + 1) * 512], acc[0:D, :])

            def proj_v(jt):
                accv = ps_proj.tile([128, 512], f32, name="ps_proj")
                for ct in range(CT):
                    nc.tensor.matmul(
                        accv[:, 0:2 * D],
                        xt[ct][:, jt * 128:(jt + 1) * 128],
                        wv_t[ct][:],
                        start=(ct == 0), stop=(ct == CT - 1),
                    )
                for h in range(2):
                    nc.vector.tensor_copy(v_cat[h][:, jt, 0:D], accv[:, h * D:(h + 1) * D])

            # ---- attention machinery ----
            # score groups: (h, isl, g) covers key tiles jt in {2g, 2g+1}
            sg_tiles = {}
            exp_tiles = {}

            def emit_scores(h, isl, g):
                sg = ps_attn.tile([128, 1024], f32, name="ps_attn")
                for t, jt in enumerate((2 * g, 2 * g + 1)):
                    nc.tensor.matmul(
                        sg[:, t * 512:(t + 1) * 512],
                        kT[h][:, jt * 128:(jt + 1) * 128],
                        qT[h][:, isl * 512:(isl + 1) * 512],
                        start=True, stop=True,
                    )
                pt = sb_p.tile([128, 1024], f32r, name="pt")
                nc.scalar.activation(pt[:], sg[:], EXP)
                sg_tiles[(h, isl, g)] = sg
                exp_tiles[(h, isl, g)] = pt

            oacc = {}

            def emit_pv(h, isl, g):
                if g == 0:
                    oacc[(h, isl)] = ps_o.tile([D + 1, 512], f32, name="ps_o")
                pt = exp_tiles.pop((h, isl, g))
                del sg_tiles[(h, isl, g)]
                Oacc = oacc[(h, isl)]
                for t, jt in enumerate((2 * g, 2 * g + 1)):
                    nc.tensor.matmul(
                        Oacc[:],
                        v_cat[h][:, jt, :],
                        pt[:, t * 512:(t + 1) * 512],
                        start=(jt == 0), stop=(jt == JT - 1),
                    )

            def emit_recip(h, isl):
                Oacc = oacc[(h, isl)]
                recip_r = sb_m.tile([1, 512], f32r, name="recip_r")
                with nc.allow_low_precision("softmax denominator reciprocal"):
                    nc.vector.reciprocal(recip_r[:], Oacc[D:D + 1, :])
                return recip_r

            def emit_bc(recip_r):
                bc_ps = ps_proj.tile([128, 512], f32, name="ps_proj")
                nc.tensor.matmul(bc_ps[0:D, :], ones1[:], recip_r[:],
                                 start=True, stop=True)
                return bc_ps

            def emit_mul(h, isl, bc_ps, copy_eng=None):
                # the DVE can read only one PSUM operand: copy the
                # unnormalized Oacc to SBUF (in parallel with the reciprocal /
                # bc broadcast), then multiply SBUF x PSUM. Also frees the
                # Oacc bank early.
                Oacc = oacc.pop((h, isl))
                ou = sb_m.tile([D, 512], f32, name="ou")
                if copy_eng is nc.scalar:
                    nc.scalar.activation(ou[:], Oacc[0:D, :], COPY)
                else:
                    nc.vector.tensor_copy(ou[:], Oacc[0:D, :])
                o = sb_o.tile([D, 512], f32r, name="o_n")
                nc.vector.tensor_mul(o[:], ou[:], bc_ps[0:D, :])
                return o

            def emit_outproj_ct(isl, ct, o0, o1, tail_i=None):
                po = ps_proj.tile([128, 512], f32, name="ps_proj")
                for h, o in ((0, o0), (1, o1)):
                    nc.tensor.matmul(
                        po[:],
                        wo_t[h][:, ct * 128:(ct + 1) * 128],
                        o[:],
                        start=(h == 0), stop=(h == 1),
                    )
                oc = sb_oc.tile([128, 512], bf16, name="oc")
                if tail_i is not None and tail_i < cfg["tail_act_copies"]:
                    nc.scalar.activation(oc[:], po[:], COPY)
                else:
                    nc.vector.tensor_copy(oc[:], po[:])
                dst = out[ct * 128:(ct + 1) * 128, isl * 512:(isl + 1) * 512]
                if tail_i is not None and tail_i < cfg["tail_swdge"]:
                    nc.gpsimd.dma_start(dst, oc[:])
                else:
                    nc.sync.dma_start(dst, oc[:])

            # ---- decoupled lead/trail emission ----
            # The lead stream (scores + exp) runs as early as possible so the
            # scalar engine — whose 68us of exp work would otherwise gate the
            # kernel tail — finishes mid-kernel. The trail stream (PV + norm +
            # out-projection + stores) lags `lag` score groups behind, living
            # off the deep sb_p pt pool; it doubles as the PE filler that
            # paces the lead to the scalar engine's throughput.
            import collections
            trail_q = collections.deque()
            lt_state = {"lead": 0, "trail": 0}
            o_norm = {}

            def pump_trail(target):
                while trail_q and lt_state["trail"] < target:
                    kind, fn = trail_q.popleft()
                    fn()
                    if kind == "pv":
                        lt_state["trail"] += 1

            def trail_norm(isl):
                r0 = emit_recip(0, isl)
                r1 = emit_recip(1, isl)
                bc0 = emit_bc(r0)
                bc1 = emit_bc(r1)
                o_norm[isl] = (emit_mul(0, isl, bc0), emit_mul(1, isl, bc1))

            def trail_outproj(isl, cts):
                o0, o1 = o_norm[isl]
                for ct in cts:
                    emit_outproj_ct(isl, ct, o0, o1)

            def emit_lead(isl, g):
                emit_scores(0, isl, g)
                emit_scores(1, isl, g)
                trail_q.append(("pv", lambda isl=isl, g=g: (
                    emit_pv(0, isl, g), emit_pv(1, isl, g))))
                if g == JG - 1 and isl < IT - 1:
                    trail_q.append(("aux", lambda isl=isl: trail_norm(isl)))
                    for cts in ((0, 1), (2, 3), (4, 5)):
                        trail_q.append(
                            ("aux", lambda isl=isl, cts=cts: trail_outproj(isl, cts)))
                lt_state["lead"] += 1
                # taper: near the end of the lead stream, drain the trail
                # deeper so the Act-paced score stalls are filled with PV work
                # and little trail remains after the last scores
                n_lead_total = IT * JG
                taper = max(0, lt_state["lead"] - (n_lead_total - cfg["lag"] + 2))
                pump_trail(lt_state["lead"] - cfg["lag"] + cfg["taper_mult"] * taper)

            # phase 1: slice-pipelined projections, isl0's lead as each key
            # slice lands
            for s in range(IT):
                trickle = cfg["warm_trickle"] if s == 0 else 0
                if s == 0:
                    dummy_mm(cfg["warm0"])
                proj_qk(0, s, wk_t, kT[0], trickle=trickle)
                proj_qk(1, s, wk_t, kT[1], trickle=trickle)
                for jt in range(4 * s, 4 * s + 4):
                    proj_v(jt)
                proj_qk(0, s, wq_t, qT[0])
                proj_qk(1, s, wq_t, qT[1])
                for g in range(2 * s, 2 * s + 2):
                    emit_lead(0, g)

            # phase 2: remaining slices' lead, trail pumping throughout
            for isl in range(1, IT):
                for g in range(JG):
                    emit_lead(isl, g)
            pump_trail(10 ** 9)

            # ---- tail: isl3 normalize + out-projection + stores ----
            # h0's half of the out-projection starts as soon as o0 is ready
            # (po tiles: 2 from ps_proj + 4 carved from the now-free ps_attn
            # tiles); h1 accumulates into them once o1 lands. Copies alternate
            # DVE/Act per chunk; the earliest stores ride the SWDGE lane.
            isl = IT - 1
            po = [None] * CT

            def mm_out(h, ct, o, stop):
                nc.tensor.matmul(
                    po[ct][:], wo_t[h][:, ct * 128:(ct + 1) * 128], o[:],
                    start=(h == 0), stop=stop,
                )

            # tail pipeline (isl3's PVs already ran in the trail): h0's exp
            # and PV finish first, so h0's normalize + opening out-proj
            # matmuls overlap h1's final exp + PV; h1 closes the accumulation
            # with copy + store chasing each closing matmul.
            r0 = emit_recip(0, isl)
            bc0 = emit_bc(r0)
            o0 = emit_mul(0, isl, bc0, copy_eng=nc.scalar)
            r1 = emit_recip(1, isl)
            CT_ORDER = (2, 3, 4, 5, 0, 1)
            for ct in CT_ORDER[:4]:
                if ct % 2 == 0:
                    big = ps_attn.tile([128, 1024], f32, name="ps_attn")
                    po[ct] = big[:, 0:512]
                else:
                    po[ct] = big[:, 512:1024]
                nc.tensor.matmul(
                    po[ct][:], wo_t[0][:, ct * 128:(ct + 1) * 128], o0[:],
                    start=True, stop=False,
                )
            bc1 = emit_bc(r1)
            o1 = emit_mul(1, isl, bc1, copy_eng=nc.scalar)
            for ct in CT_ORDER[4:]:
                po[ct] = ps_o.tile([128, 512], f32, name="ps_o")
                nc.tensor.matmul(
                    po[ct][:], wo_t[0][:, ct * 128:(ct + 1) * 128], o0[:],
                    start=True, stop=False,
                )
            for i, ct in enumerate(CT_ORDER):
                nc.tensor.matmul(
                    po[ct][:], wo_t[1][:, ct * 128:(ct + 1) * 128], o1[:],
                    start=False, stop=True,
                )
                oc = sb_oc.tile([128, 512], bf16, name="oc")
                if i % 2 == 0:
                    nc.scalar.activation(oc[:], po[ct][:], COPY)
                else:
                    nc.vector.tensor_copy(oc[:], po[ct][:])
                dst = out[ct * 128:(ct + 1) * 128, isl * 512:(isl + 1) * 512]
                if i < cfg["tail_swdge"]:
                    nc.gpsimd.dma_start(dst, oc[:])
                else:
                    nc.sync.dma_start(dst, oc[:])

    _split_sync_waits(nc, mybir)
    return nc


class _Runner:
    """Compile once, run many. Mirrors run_bass_via_pjrt's multi-core path but
    keeps the jitted executable cached across calls."""

    def __init__(self, cfg=None):
        import jax
        import concourse.mybir as mybir
        from concourse import bass2jax
        from jax.sharding import Mesh, PartitionSpec
        from jax.experimental.shard_map import shard_map

        self.jax = jax
        nc = _build_nc(cfg)
        self.nc = nc
        bass2jax.install_neuronx_cc_hook()

        in_names, out_names, out_avals = [], [], []
        for alloc in nc.m.functions[0].allocations:
            if not isinstance(alloc, mybir.MemoryLocationSet):
                continue
            name = alloc.memorylocations[0].name
            if alloc.kind == "ExternalInput":
                if nc.partition_id_tensor is None or name != nc.partition_id_tensor.name:
                    in_names.append(name)
            elif alloc.kind == "ExternalOutput":
                out_names.append(name)
                out_avals.append(
                    jax.core.ShapedArray(tuple(alloc.tensor_shape), mybir.dt.np(alloc.dtype))
                )
        self.in_names = in_names
        self.out_names = out_names
        partition_name = nc.partition_id_tensor.name if nc.partition_id_tensor else None
        all_names = tuple(in_names + out_names + ([partition_name] if partition_name else []))

        def _body(*args):
            operands = list(args)
            if partition_name is not None:
                operands.append(bass2jax.partition_id_tensor())
            outs = bass2jax._bass_exec_p.bind(
                *operands,
                out_avals=tuple(out_avals),
                in_names=all_names,
                out_names=tuple(out_names),
                lowering_input_output_aliases=(),
                sim_require_finite=True,
                sim_require_nnan=True,
                nc=nc,
            )
            return tuple(outs)

        devices = jax.devices()[:N_CORES]
        mesh = Mesh(np.asarray(devices), ("core",))
        n_all = len(in_names) + len(out_names)
        self.sharded = jax.jit(
            shard_map(
                _body,
                mesh=mesh,
                in_specs=(PartitionSpec("core"),) * n_all,
                out_specs=(PartitionSpec("core"),) * len(out_names),
                check_rep=False,
            ),
            keep_unused=True,
        )
        self.out_shapes = [tuple(a.shape) for a in out_avals]
        self.out_dtypes = [a.dtype for a in out_avals]

    def run(self, in_maps):
        concat_in = [
            np.concatenate([np.asarray(in_maps[c][n]) for c in range(N_CORES)], axis=0)
            for n in self.in_names
        ]
        concat_zero = [
            np.zeros((N_CORES * s[0], *s[1:]), d)
            for s, d in zip(self.out_shapes, self.out_dtypes)
        ]
        outs = self.sharded(*concat_in, *concat_zero)
        self.jax.block_until_ready(outs)
        return [
            {
                n: np.asarray(outs[i]).reshape(N_CORES, *self.out_shapes[i])[c]
                for i, n in enumerate(self.out_names)
            }
            for c in range(N_CORES)
        ]


def _get_runner():
    global _RUNNER
    if _RUNNER is None:
        _RUNNER = _Runner()
    return _RUNNER


def _pack_w(w):
    """(768, 192) -> (128, 6*192) partition-major: out[p, ct*192+j] = w[ct*128+p, j]."""
    return np.ascontiguousarray(
        w.reshape(CT, 128, 2 * D).transpose(1, 0, 2).reshape(128, CT * 2 * D)
    )


def _shard_inputs(inputs, W_qkv, W_out):
    import ml_dtypes

    bf16 = ml_dtypes.bfloat16
    in_maps = []
    for core in range(N_CORES):
        b, g = divmod(core, 4)
        cols = slice(g * 2 * D, (g + 1) * 2 * D)
        wo = W_out[cols, :]  # (192, 768)
        wo_packed = np.ascontiguousarray(
            wo.reshape(2, D, C).transpose(1, 0, 2).reshape(D, 2 * C)
        )
        in_maps.append({
            "x": np.ascontiguousarray(inputs[b]).astype(bf16),
            "wq": _pack_w(W_qkv[:, cols]).astype(bf16),
            "wk": _pack_w(W_qkv[:, 768:][:, cols]).astype(bf16),
            "wv": _pack_w(W_qkv[:, 1536:][:, cols]).astype(bf16),
            "wo": wo_packed,
        })
    return in_maps


def kernel(inputs, W_qkv, W_out):
    inputs = np.asarray(inputs, dtype=np.float32)
    W_qkv = np.asarray(W_qkv, dtype=np.float32)
    W_out = np.asarray(W_out, dtype=np.float32)
    runner = _get_runner()
    results = runner.run(_shard_inputs(inputs, W_qkv, W_out))
    out = np.zeros((B, C, S), np.float32)
    for core in range(N_CORES):
        out[core // 4] += results[core]["out"].astype(np.float32)
    return out


# revision 61
# speedup vs baseline: 1.1885x; 1.0288x over previous
"""Multi-head attention (b=2, c=768, s=2048, 8 heads, d=96) on 8 TRN2 NeuronCores.

Sharding: batch x head-group tensor parallel. Core i handles batch i//4 and
heads {2*(i%4), 2*(i%4)+1}; the host sums the 4 partial outputs per batch
element (the all-reduce of the sharding hint, done host-side since the kernel
returns full outputs anyway).

v2 schedule (110.7us TimelineSim vs the v1 baseline's 127.9us; measured HW
rel err ~9e-3 against the f32 reference, tolerance 2e-2):
  - x and the QKV weights are uploaded as bf16 (host-converted, host-packed
    partition-major so every DMA descriptor is a full 2KB+ row): halves the
    input DMA stream and lets the v-projection run at N=192 without the f32r
    N>=256 zero-padding. Scores / PV / out-projection stay f32r.
  - output stores are bf16 (host upcasts and sums the per-core partials):
    halves store traffic so the final-slice store burst shrinks.
  - PE warmup: dummy matmuls on a zero tile keep the tensor engine's p-state
    ramp warm while the first x chunks stream in; slice 0 loads as 512-wide
    chunks, the rest as 768-wide chunks (HWDGE generation, 625ns per DMA, is
    the input-stream limiter once transfers are bf16).
  - attention is emitted as a decoupled lead stream (scores + exp, as early
    as each key slice's projections land — exp starts ~8us in) and a trail
    stream (PV + normalize + out-projection + stores) that lags `lag` score
    groups behind, buffered in the deep sb_p pt pool. The trail doubles as
    the PE filler that paces the lead to the scalar engine's exp throughput
    (the attention-phase bottleneck at ~66us busy), so the scalar engine
    finishes its exp queue mid-kernel instead of gating the tail.
  - normalize: the softmax denominator row (PV's ones-column) is
    reciprocal'd on DVE, broadcast across partitions via a K=1 matmul, and
    multiplied against an SBUF copy of the unnormalized Oacc (the DVE can
    read only one PSUM operand per instruction).
  - tail: h0's normalize + opening out-proj matmuls overlap h1's final
    exp/PV; po tiles borrow the freed ps_attn banks; copies alternate
    Act/DVE per chunk and the first stores ride the gpsimd SWDGE lane so
    descriptor generation runs in two lanes.
"""

import numpy as np

N_CORES = 8
B, C, S = 2, 768, 2048
H, D = 8, 96
CT = C // 128          # 6 c-tiles
IT = S // 512          # 4 query slices
JT = S // 128          # 16 key tiles
JG = JT // 2           # 8 exp groups of 2 key tiles

_RUNNER = None


def _split_sync_waits(nc, mybir, max_waits=1):
    """This walrus build rejects instructions carrying more than one sem wait
    (setupSyncWait: 'Too many sync wait commands'). Split excess waits onto
    same-engine NoOps inserted immediately before the instruction."""
    for bb in nc.main_func.blocks:
        insts = bb.instructions
        i = 0
        while i < len(insts):
            inst = insts[i]
            si = inst.sync_info
            if si is not None and si.on_wait and len(si.on_wait) > max_waits:
                waits = list(si.on_wait)
                keep = waits[-max_waits:]
                extra = waits[:-max_waits]
                pos = i
                while extra:
                    chunk, extra = extra[:max_waits], extra[max_waits:]
                    nop = mybir.InstNoOp(
                        name=nc.get_next_instruction_name(),
                        sync_info=mybir.SyncInfo(on_wait=chunk, on_update=[]),
                        engine=inst.engine,
                        bass_nofuse=True,
                    )
                    insts.insert(pos, nop)
                    pos += 1
                    i += 1
                si.on_wait = keep
            i += 1


DEFAULT_CFG = dict(
    warm0=22,            # warmup dummy matmuls before the first projection
    warm_trickle=1,      # dummies interleaved after each slice-0 c-tile matmul
    lag=6,               # score groups the PV/outproj trail runs behind
    taper_mult=1,        # how aggressively the trail drains near the end
    gate_wqv=2,          # x-s0 chunk whose DMA gates the wq/wv SWDGE loads
    gate_wo=1,           # x slice whose last chunk gates the wo SWDGE load
    tail_act_copies=3,   # of the 6 tail out copies, how many go on Act
    tail_swdge=2,        # of the 6 tail stores, how many go on the gpsimd queue
    loop_n=1,
)


def _build_nc(cfg=None):
    import concourse.bass as bass
    import concourse.tile as tile
    import concourse.mybir as mybir
    from concourse.tile import add_dep_helper

    cfg = {**DEFAULT_CFG, **(cfg or {})}

    f32 = mybir.dt.float32
    f32r = mybir.dt.float32r
    bf16 = mybir.dt.bfloat16
    EXP = mybir.ActivationFunctionType.Exp
    COPY = mybir.ActivationFunctionType.Copy

    # weights arrive host-packed partition-major so every DMA descriptor is a
    # full 2KB+ contiguous row (small descriptors pay a 2x DMA penalty):
    #   wq/wk/wv: [128, ct*192+j] = W[ct*128+p, j]   (bf16)
    #   wo:       [96, h*768+c]  = W_out[h*96+p, c]  (f32)
    nc = bass.Bass(num_devices=N_CORES)
    x = nc.declare_dram_parameter("x", [C, S], bf16, isOutput=False)
    wqk = nc.declare_dram_parameter("wqk", [128, CT * 4 * D], bf16, isOutput=False)
    wv = nc.declare_dram_parameter("wv", [128, CT * 2 * D], bf16, isOutput=False)
    wo = nc.declare_dram_parameter("wo", [D, 2 * C], f32, isOutput=False)
    out = nc.declare_dram_parameter("out", [C, S], bf16, isOutput=True)

    with tile.TileContext(nc) as tc:
        with (
            tc.tile_pool(name="sb_x", bufs=1) as sb_x,
            tc.tile_pool(name="sb_w", bufs=1) as sb_w,
            tc.tile_pool(name="sb_qk", bufs=1) as sb_qk,
            tc.tile_pool(name="sb_v", bufs=1) as sb_v,
            tc.tile_pool(name="sb_p", bufs=cfg["lag"] + 2) as sb_p,
            tc.tile_pool(name="sb_o", bufs=3) as sb_o,
            tc.tile_pool(name="sb_m", bufs=6) as sb_m,
            tc.tile_pool(name="sb_oc", bufs=8) as sb_oc,
            tc.tile_pool(name="ps_proj", bufs=2, space="PSUM") as ps_proj,
            tc.tile_pool(name="ps_attn", bufs=2, space="PSUM") as ps_attn,
            tc.tile_pool(name="ps_o", bufs=2, space="PSUM") as ps_o,
        ):
          import contextlib
          loop_ctx = tc.For_i(0, cfg["loop_n"], 1) if cfg["loop_n"] > 1 else contextlib.nullcontext()
          with loop_ctx:
            # bf16 memset works directly (f32r doesn't), saving the
            # f32->f32r copy on the warmup critical path.
            # zr is a single partition row: the warmup matmul contracts K=1.
            zr = sb_w.tile([1, 64], bf16, name="zr")
            nc.vector.memset(zr[:], 0.0)
            cone = sb_w.tile([128, 96], f32, name="cone")
            nc.vector.memset(cone[:], 1.0)
            ones1 = sb_w.tile([1, D], f32r, name="ones1")
            nc.vector.tensor_copy(ones1[:], cone[0:1, :])

            def dummy_mm(n=1):
                """PE p-state warmers: small f32r matmul on the zero tile."""
                for _ in range(n):
                    dps = ps_proj.tile([128, 512], f32, name="ps_proj")
                    nc.tensor.matmul(dps[0:64, 0:64], zr[:], zr[:],
                                     start=True, stop=True)

            # ---- input DMAs ----
            # slice 0 as per-(ct,512) chunks for fast availability; the rest
            # as two (128, 768) DMAs per c-tile (fewer HWDGE generations than
            # per-slice chunks, smoother arrival than one wide DMA)
            xt_s0 = {ct: sb_x.tile([128, 512], bf16, name=f"xt{ct}_0")
                     for ct in range(CT)}
            xt_rest = {ct: sb_x.tile([128, 3 * 512], bf16, name=f"xt{ct}_r")
                       for ct in range(CT)}
            x_dmas = {}

            def load_x_slice0():
                for ct in range(CT):
                    x_dmas[(ct, 0)] = nc.sync.dma_start(
                        xt_s0[ct][:],
                        x[ct * 128:(ct + 1) * 128, 0:512],
                    )

            def load_x_rest(ct, half):
                a, b = (0, 768) if half == 0 else (768, 1536)
                d = nc.sync.dma_start(
                    xt_rest[ct][:, a:b],
                    x[ct * 128:(ct + 1) * 128, 512 + a:512 + b],
                )
                # half 0 covers slice 1 and half of slice 2; half 1 the rest
                if half == 0:
                    x_dmas[(ct, 1)] = d
                else:
                    x_dmas[(ct, 2)] = d
                    x_dmas[(ct, 3)] = d

            class _XtView:
                def __init__(self, ct):
                    self.ct = ct
                def __getitem__(self, key):
                    rows, cols = key
                    a, b = cols.start or 0, cols.stop
                    assert b - a <= 512
                    if b <= 512:
                        return xt_s0[self.ct][rows, a:b]
                    assert a >= 512
                    return xt_rest[self.ct][rows, a - 512:b - 512]

            xt = [_XtView(ct) for ct in range(CT)]

            # weights: the packed q/k matrix on SWDGE immediately (its
            # transfer slots between the first x chunks); wv gated on a later
            # x-s0 chunk so slice 0 completes first; wo gated on x-rest.
            tqk = sb_w.tile([128, CT * 4 * D], bf16, name="wqk")
            nc.gpsimd.dma_start(tqk[:], wqk[:])
            wqk_t = [tqk[:, ct * 4 * D:(ct + 1) * 4 * D] for ct in range(CT)]

            load_x_slice0()

            tv = sb_w.tile([128, CT * 2 * D], bf16, name="wv")
            d_wv = nc.gpsimd.dma_start(tv[:], wv[:])
            gate = x_dmas[(cfg["gate_wqv"], 0)]
            add_dep_helper(d_wv.ins, gate.ins, sync=True, reason="wv after x s0")
            wv_t = [tv[:, ct * 2 * D:(ct + 1) * 2 * D] for ct in range(CT)]

            for ct in range(CT):
                load_x_rest(ct, 0)
            for ct in range(CT):
                load_x_rest(ct, 1)

            two = sb_w.tile([D, 2 * C], f32r, name="wo")
            d_wo = nc.gpsimd.dma_start(two[:], wo[:].bitcast(f32r))
            add_dep_helper(d_wo.ins, x_dmas[(CT - 1, cfg["gate_wo"])].ins,
                           sync=True, reason="wo after x")
            wo_t = [two[:, h * C:(h + 1) * C] for h in range(2)]

            # ---- persistent compute tiles ----
            # The packed projection yields 3 full 128-row tiles per slice:
            #   stg0 rows 0-95 = q0, 96-127 = q1 d0-31
            #   stg1 rows 0-63 = q1 d32-95, 64-127 = k0 d0-63
            #   stg2 rows 0-31 = k0 d64-95, 32-127 = k1
            # q0 is used in place (stg0 view); q1/k0/k1 are reassembled by
            # partition-shifted gpsimd copies on the otherwise idle pool.
            stg = [sb_qk.tile([128, S], f32r, name=f"stg{t}") for t in range(3)]
            # h1 runs a K=128 contraction: kT1 is stg2 in place (k1 at rows
            # 32-127, k0 spill at rows 0-31) and qT1 holds q1 at rows 32-127
            # with rows 0-31 zeroed, so the mismatched rows contribute
            # finite * 0 = 0.
            qT1full = sb_qk.tile([128, S], f32r, name="qT1")
            zq = sb_w.tile([32, S], f32, name="zq")
            nc.vector.memset(zq[:], 0.0)
            nc.vector.tensor_copy(qT1full[0:32, :], zq[:])
            qT = [stg[0][0:D, :], qT1full[:]]
            kT = [sb_qk.tile([D, S], f32r, name="kT0"), stg[2][:]]
            v_cat = [sb_v.tile([128, JT, D + 1], f32r, name=f"v{h}") for h in range(2)]
            for h in range(2):
                nc.vector.tensor_copy(v_cat[h][:, :, D], cone[:, 0:JT])

            def proj_qk3(s, trickle=0):
                sl = slice(s * 512, (s + 1) * 512)
                for t in range(3):
                    acc = ps_proj.tile([128, 512], f32, name="ps_proj")
                    for ct in range(CT):
                        nc.tensor.matmul(
                            acc[:],
                            wqk_t[ct][:, t * 128:(t + 1) * 128],
                            xt[ct][:, s * 512:(s + 1) * 512],
                            start=(ct == 0), stop=(ct == CT - 1),
                        )
                        if trickle and t == 0:
                            dummy_mm(trickle)
                    nc.vector.tensor_copy(stg[t][:, sl], acc[:])
                # partition-shifted reassembly in 32-partition pieces (the
                # compiler limits partition windows to 32 from a 32-aligned
                # start); only k here — the attention lead needs kT; qT1 for
                # slice s isn't consumed until the lead reaches isl s, so its
                # copies are deferred off the pool's critical path
                nc.gpsimd.tensor_copy(kT[0][0:64, sl], stg[1][64:128, sl])
                nc.gpsimd.tensor_copy(kT[0][64:D, sl], stg[2][0:32, sl])

            def emit_q_copies(s):
                # q1 into rows 32-127 of qT1full (matching kT1's d -> d+32)
                sl = slice(s * 512, (s + 1) * 512)
                nc.gpsimd.tensor_copy(qT1full[32:64, sl], stg[0][D:128, sl])
                nc.gpsimd.tensor_copy(qT1full[64:128, sl], stg[1][0:64, sl])

            def proj_v(jt):
                accv = ps_proj.tile([128, 512], f32, name="ps_proj")
                for ct in range(CT):
                    nc.tensor.matmul(
                        accv[:, 0:2 * D],
                        xt[ct][:, jt * 128:(jt + 1) * 128],
                        wv_t[ct][:],
                        start=(ct == 0), stop=(ct == CT - 1),
                    )
                for h in range(2):
                    nc.vector.tensor_copy(v_cat[h][:, jt, 0:D], accv[:, h * D:(h + 1) * D])

            # ---- attention machinery ----
            # score groups: (h, isl, g) covers key tiles jt in {2g, 2g+1}
            sg_tiles = {}
            exp_tiles = {}

            def emit_scores(h, isl, g):
                sg = ps_attn.tile([128, 1024], f32, name="ps_attn")
                for t, jt in enumerate((2 * g, 2 * g + 1)):
                    nc.tensor.matmul(
                        sg[:, t * 512:(t + 1) * 512],
                        kT[h][:, jt * 128:(jt + 1) * 128],
                        qT[h][:, isl * 512:(isl + 1) * 512],
                        start=True, stop=True,
                    )
                pt = sb_p.tile([128, 1024], f32r, name="pt")
                nc.scalar.activation(pt[:], sg[:], EXP)
                sg_tiles[(h, isl, g)] = sg
                exp_tiles[(h, isl, g)] = pt

            oacc = {}

            def emit_pv(h, isl, g):
                if g == 0:
                    oacc[(h, isl)] = ps_o.tile([D + 1, 512], f32, name="ps_o")
                pt = exp_tiles.pop((h, isl, g))
                del sg_tiles[(h, isl, g)]
                Oacc = oacc[(h, isl)]
                for t, jt in enumerate((2 * g, 2 * g + 1)):
                    nc.tensor.matmul(
                        Oacc[:],
                        v_cat[h][:, jt, :],
                        pt[:, t * 512:(t + 1) * 512],
                        start=(jt == 0), stop=(jt == JT - 1),
                    )

            def emit_recip(h, isl):
                Oacc = oacc[(h, isl)]
                recip_r = sb_m.tile([1, 512], f32r, name="recip_r")
                with nc.allow_low_precision("softmax denominator reciprocal"):
                    nc.vector.reciprocal(recip_r[:], Oacc[D:D + 1, :])
                return recip_r

            def emit_bc(recip_r):
                bc_ps = ps_proj.tile([128, 512], f32, name="ps_proj")
                nc.tensor.matmul(bc_ps[0:D, :], ones1[:], recip_r[:],
                                 start=True, stop=True)
                return bc_ps

            def emit_mul(h, isl, bc_ps, copy_eng=None):
                # the DVE can read only one PSUM operand: copy the
                # unnormalized Oacc to SBUF (in parallel with the reciprocal /
                # bc broadcast), then multiply SBUF x PSUM. Also frees the
                # Oacc bank early.
                Oacc = oacc.pop((h, isl))
                ou = sb_m.tile([D, 512], f32, name="ou")
                if copy_eng is nc.scalar:
                    nc.scalar.activation(ou[:], Oacc[0:D, :], COPY)
                else:
                    nc.vector.tensor_copy(ou[:], Oacc[0:D, :])
                o = sb_o.tile([D, 512], f32r, name="o_n")
                nc.vector.tensor_mul(o[:], ou[:], bc_ps[0:D, :])
                return o

            def emit_outproj_ct(isl, ct, o0, o1, tail_i=None):
                po = ps_proj.tile([128, 512], f32, name="ps_proj")
                for h, o in ((0, o0), (1, o1)):
                    nc.tensor.matmul(
                        po[:],
                        wo_t[h][:, ct * 128:(ct + 1) * 128],
                        o[:],
                        start=(h == 0), stop=(h == 1),
                    )
                oc = sb_oc.tile([128, 512], bf16, name="oc")
                if tail_i is not None and tail_i < cfg["tail_act_copies"]:
                    nc.scalar.activation(oc[:], po[:], COPY)
                else:
                    nc.vector.tensor_copy(oc[:], po[:])
                dst = out[ct * 128:(ct + 1) * 128, isl * 512:(isl + 1) * 512]
                if tail_i is not None and tail_i < cfg["tail_swdge"]:
                    nc.gpsimd.dma_start(dst, oc[:])
                else:
                    nc.sync.dma_start(dst, oc[:])

            # ---- decoupled lead/trail emission ----
            # The lead stream (scores + exp) runs as early as possible so the
            # scalar engine — whose 68us of exp work would otherwise gate the
            # kernel tail — finishes mid-kernel. The trail stream (PV + norm +
            # out-projection + stores) lags `lag` score groups behind, living
            # off the deep sb_p pt pool; it doubles as the PE filler that
            # paces the lead to the scalar engine's throughput.
            import collections
            trail_q = collections.deque()
            lt_state = {"lead": 0, "trail": 0}
            o_norm = {}

            def pump_trail(target):
                while trail_q and lt_state["trail"] < target:
                    kind, fn = trail_q.popleft()
                    fn()
                    if kind == "pv":
                        lt_state["trail"] += 1

            def trail_norm(isl):
                r0 = emit_recip(0, isl)
                r1 = emit_recip(1, isl)
                bc0 = emit_bc(r0)
                bc1 = emit_bc(r1)
                o_norm[isl] = (emit_mul(0, isl, bc0), emit_mul(1, isl, bc1))

            def trail_outproj(isl, cts):
                o0, o1 = o_norm[isl]
                for ct in cts:
                    emit_outproj_ct(isl, ct, o0, o1)

            def emit_lead(isl, g):
                emit_scores(0, isl, g)
                emit_scores(1, isl, g)
                trail_q.append(("pv", lambda isl=isl, g=g: (
                    emit_pv(0, isl, g), emit_pv(1, isl, g))))
                if g == JG - 1 and isl < IT - 1:
                    trail_q.append(("aux", lambda isl=isl: trail_norm(isl)))
                    for cts in ((0, 1), (2, 3), (4, 5)):
                        trail_q.append(
                            ("aux", lambda isl=isl, cts=cts: trail_outproj(isl, cts)))
                lt_state["lead"] += 1
                # taper: near the end of the lead stream, drain the trail
                # deeper so the Act-paced score stalls are filled with PV work
                # and little trail remains after the last scores
                n_lead_total = IT * JG
                taper = max(0, lt_state["lead"] - (n_lead_total - cfg["lag"] + 2))
                pump_trail(lt_state["lead"] - cfg["lag"] + cfg["taper_mult"] * taper)

            # phase 1: slice-pipelined projections, isl0's lead as each key
            # slice lands
            for s in range(IT):
                trickle = cfg["warm_trickle"] if s == 0 else 0
                if s == 0:
                    dummy_mm(cfg["warm0"])
                proj_qk3(s, trickle=trickle)
                if s == 0:
                    emit_q_copies(0)
                for jt in range(4 * s, 4 * s + 4):
                    proj_v(jt)
                if s > 0:
                    emit_q_copies(s)
                for g in range(2 * s, 2 * s + 2):
                    emit_lead(0, g)

            # phase 2: remaining slices' lead, trail pumping throughout
            for isl in range(1, IT):
                for g in range(JG):
                    emit_lead(isl, g)
            pump_trail(10 ** 9)

            # ---- tail: isl3 normalize + out-projection + stores ----
            # h0's half of the out-projection starts as soon as o0 is ready
            # (po tiles: 2 from ps_proj + 4 carved from the now-free ps_attn
            # tiles); h1 accumulates into them once o1 lands. Copies alternate
            # DVE/Act per chunk; the earliest stores ride the SWDGE lane.
            isl = IT - 1
            po = [None] * CT

            def mm_out(h, ct, o, stop):
                nc.tensor.matmul(
                    po[ct][:], wo_t[h][:, ct * 128:(ct + 1) * 128], o[:],
                    start=(h == 0), stop=stop,
                )

            # tail pipeline (isl3's PVs already ran in the trail): h0's exp
            # and PV finish first, so h0's normalize + opening out-proj
            # matmuls overlap h1's final exp + PV; h1 closes the accumulation
            # with copy + store chasing each closing matmul.
            r0 = emit_recip(0, isl)
            bc0 = emit_bc(r0)
            o0 = emit_mul(0, isl, bc0, copy_eng=nc.scalar)
            r1 = emit_recip(1, isl)
            CT_ORDER = (2, 3, 4, 5, 0, 1)
            for ct in CT_ORDER[:4]:
                if ct % 2 == 0:
                    big = ps_attn.tile([128, 1024], f32, name="ps_attn")
                    po[ct] = big[:, 0:512]
                else:
                    po[ct] = big[:, 512:1024]
                nc.tensor.matmul(
                    po[ct][:], wo_t[0][:, ct * 128:(ct + 1) * 128], o0[:],
                    start=True, stop=False,
                )
            bc1 = emit_bc(r1)
            o1 = emit_mul(1, isl, bc1, copy_eng=nc.scalar)
            for ct in CT_ORDER[4:]:
                po[ct] = ps_o.tile([128, 512], f32, name="ps_o")
                nc.tensor.matmul(
                    po[ct][:], wo_t[0][:, ct * 128:(ct + 1) * 128], o0[:],
                    start=True, stop=False,
                )
            for i, ct in enumerate(CT_ORDER):
                nc.tensor.matmul(
                    po[ct][:], wo_t[1][:, ct * 128:(ct + 1) * 128], o1[:],
                    start=False, stop=True,
                )
                oc = sb_oc.tile([128, 512], bf16, name="oc")
                if i % 2 == 0:
                    nc.scalar.activation(oc[:], po[ct][:], COPY)
                else:
                    nc.vector.tensor_copy(oc[:], po[ct][:])
                dst = out[ct * 128:(ct + 1) * 128, isl * 512:(isl + 1) * 512]
                if i < cfg["tail_swdge"]:
                    nc.gpsimd.dma_start(dst, oc[:])
                else:
                    nc.sync.dma_start(dst, oc[:])

    _split_sync_waits(nc, mybir)
    return nc


class _Runner:
    """Compile once, run many. Mirrors run_bass_via_pjrt's multi-core path but
    keeps the jitted executable cached across calls."""

    def __init__(self, cfg=None):
        import jax
        import concourse.mybir as mybir
        from concourse import bass2jax
        from jax.sharding import Mesh, PartitionSpec
        from jax.experimental.shard_map import shard_map

        self.jax = jax
        nc = _build_nc(cfg)
        self.nc = nc
        bass2jax.install_neuronx_cc_hook()

        in_names, out_names, out_avals = [], [], []
        for alloc in nc.m.functions[0].allocations:
            if not isinstance(alloc, mybir.MemoryLocationSet):
                continue
            name = alloc.memorylocations[0].name
            if alloc.kind == "ExternalInput":
                if nc.partition_id_tensor is None or name != nc.partition_id_tensor.name:
                    in_names.append(name)
            elif alloc.kind == "ExternalOutput":
                out_names.append(name)
                out_avals.append(
                    jax.core.ShapedArray(tuple(alloc.tensor_shape), mybir.dt.np(alloc.dtype))
                )
        self.in_names = in_names
        self.out_names = out_names
        partition_name = nc.partition_id_tensor.name if nc.partition_id_tensor else None
        all_names = tuple(in_names + out_names + ([partition_name] if partition_name else []))

        def _body(*args):
            operands = list(args)
            if partition_name is not None:
                operands.append(bass2jax.partition_id_tensor())
            outs = bass2jax._bass_exec_p.bind(
                *operands,
                out_avals=tuple(out_avals),
                in_names=all_names,
                out_names=tuple(out_names),
                lowering_input_output_aliases=(),
                sim_require_finite=True,
                sim_require_nnan=True,
                nc=nc,
            )
            return tuple(outs)

        devices = jax.devices()[:N_CORES]
        mesh = Mesh(np.asarray(devices), ("core",))
        n_all = len(in_names) + len(out_names)
        self.sharded = jax.jit(
            shard_map(
                _body,
                mesh=mesh,
                in_specs=(PartitionSpec("core"),) * n_all,
                out_specs=(PartitionSpec("core"),) * len(out_names),
                check_rep=False,
            ),
            keep_unused=True,
        )
        self.out_shapes = [tuple(a.shape) for a in out_avals]
        self.out_dtypes = [a.dtype for a in out_avals]

    def run(self, in_maps):
        concat_in = [
            np.concatenate([np.asarray(in_maps[c][n]) for c in range(N_CORES)], axis=0)
            for n in self.in_names
        ]
        concat_zero = [
            np.zeros((N_CORES * s[0], *s[1:]), d)
            for s, d in zip(self.out_shapes, self.out_dtypes)
        ]
        outs = self.sharded(*concat_in, *concat_zero)
        self.jax.block_until_ready(outs)
        return [
            {
                n: np.asarray(outs[i]).reshape(N_CORES, *self.out_shapes[i])[c]
                for i, n in enumerate(self.out_names)
            }
            for c in range(N_CORES)
        ]


def _get_runner():
    global _RUNNER
    if _RUNNER is None:
        _RUNNER = _Runner()
    return _RUNNER


def _pack_w(w):
    """(768, 192) -> (128, 6*192) partition-major: out[p, ct*192+j] = w[ct*128+p, j]."""
    return np.ascontiguousarray(
        w.reshape(CT, 128, 2 * D).transpose(1, 0, 2).reshape(128, CT * 2 * D)
    )


def _shard_inputs(inputs, W_qkv, W_out):
    import ml_dtypes

    bf16 = ml_dtypes.bfloat16
    in_maps = []
    for core in range(N_CORES):
        b, g = divmod(core, 4)
        cols = slice(g * 2 * D, (g + 1) * 2 * D)
        wo = W_out[cols, :]  # (192, 768)
        wo_packed = np.ascontiguousarray(
            wo.reshape(2, D, C).transpose(1, 0, 2).reshape(D, 2 * C)
        )
        # packed q/k: per c-tile the 384 output dims are [q0|q1|k0|k1]
        q = W_qkv[:, cols].reshape(CT, 128, 2 * D)
        k = W_qkv[:, 768:][:, cols].reshape(CT, 128, 2 * D)
        wqk = np.concatenate([q, k], axis=2)  # (CT, 128, 384)
        wqk = np.ascontiguousarray(
            wqk.transpose(1, 0, 2).reshape(128, CT * 4 * D)
        )
        in_maps.append({
            "x": np.ascontiguousarray(inputs[b]).astype(bf16),
            "wqk": wqk.astype(bf16),
            "wv": _pack_w(W_qkv[:, 1536:][:, cols]).astype(bf16),
            "wo": wo_packed,
        })
    return in_maps


def kernel(inputs, W_qkv, W_out):
    inputs = np.asarray(inputs, dtype=np.float32)
    W_qkv = np.asarray(W_qkv, dtype=np.float32)
    W_out = np.asarray(W_out, dtype=np.float32)
    runner = _get_runner()
    results = runner.run(_shard_inputs(inputs, W_qkv, W_out))
    out = np.zeros((B, C, S), np.float32)
    for core in range(N_CORES):
        out[core // 4] += results[core]["out"].astype(np.float32)
    return out


# revision 65
# speedup vs baseline: 1.1924x; 1.0032x over previous
"""Multi-head attention (b=2, c=768, s=2048, 8 heads, d=96) on 8 TRN2 NeuronCores.

Sharding: batch x head-group tensor parallel. Core i handles batch i//4 and
heads {2*(i%4), 2*(i%4)+1}; the host sums the 4 partial outputs per batch
element (the all-reduce of the sharding hint, done host-side since the kernel
returns full outputs anyway).

v2 schedule (107.6us TimelineSim vs the v1 baseline's 127.9us; measured HW
rel err ~9e-3 against the f32 reference, tolerance 2e-2):
  - x and the QKV weights are uploaded as bf16 (host-converted, host-packed
    partition-major so every DMA descriptor is a full 2KB+ row): halves the
    input DMA stream and lets the v-projection run at N=192 without the f32r
    N>=256 zero-padding. Scores / PV / out-projection stay f32r.
  - the q/k weights pack as [q0|q1|k0|k1] per c-tile so the projection runs
    as 3 full 128-row matmul tiles per slice instead of 4 x 96-row ones
    (18 vs 24 matmuls, -5us PE). q0 is consumed in place; k0 is reassembled
    by two partition-shifted gpsimd copies; h1 contracts over K=128 with
    kT1 read in place from the staging tile (k1 at rows 32-127) against a
    q1 tile whose rows 0-31 are zeroed, so the spill rows contribute 0.
  - output stores are bf16 (host upcasts and sums the per-core partials):
    halves store traffic so the final-slice store burst shrinks.
  - PE warmup: dummy matmuls on a zero tile keep the tensor engine's p-state
    ramp warm while the first x chunks stream in; slice 0 loads as 512-wide
    chunks, the rest as 768-wide chunks (HWDGE generation, 625ns per DMA, is
    the input-stream limiter once transfers are bf16).
  - attention is emitted as a decoupled lead stream (scores + exp, as early
    as each key slice's projections land — exp starts ~8us in) and a trail
    stream (PV + normalize + out-projection + stores) that lags `lag` score
    groups behind, buffered in the deep sb_p pt pool. The trail doubles as
    the PE filler that paces the lead to the scalar engine's exp throughput
    (the attention-phase bottleneck at ~66us busy), so the scalar engine
    finishes its exp queue mid-kernel instead of gating the tail.
  - normalize: the softmax denominator row (PV's ones-column) is
    reciprocal'd on DVE, broadcast across partitions via a K=1 matmul, and
    multiplied against an SBUF copy of the unnormalized Oacc (the DVE can
    read only one PSUM operand per instruction).
  - tail: h0's normalize + opening out-proj matmuls overlap h1's final
    exp/PV; po tiles borrow the freed ps_attn banks; copies alternate
    Act/DVE per chunk and the first stores ride the gpsimd SWDGE lane so
    descriptor generation runs in two lanes.
"""

import numpy as np

N_CORES = 8
B, C, S = 2, 768, 2048
H, D = 8, 96
CT = C // 128          # 6 c-tiles
IT = S // 512          # 4 query slices
JT = S // 128          # 16 key tiles
JG = JT // 2           # 8 exp groups of 2 key tiles

_RUNNER = None


def _split_sync_waits(nc, mybir, max_waits=1):
    """This walrus build rejects instructions carrying more than one sem wait
    (setupSyncWait: 'Too many sync wait commands'). Split excess waits onto
    same-engine NoOps inserted immediately before the instruction."""
    for bb in nc.main_func.blocks:
        insts = bb.instructions
        i = 0
        while i < len(insts):
            inst = insts[i]
            si = inst.sync_info
            if si is not None and si.on_wait and len(si.on_wait) > max_waits:
                waits = list(si.on_wait)
                keep = waits[-max_waits:]
                extra = waits[:-max_waits]
                pos = i
                while extra:
                    chunk, extra = extra[:max_waits], extra[max_waits:]
                    nop = mybir.InstNoOp(
                        name=nc.get_next_instruction_name(),
                        sync_info=mybir.SyncInfo(on_wait=chunk, on_update=[]),
                        engine=inst.engine,
                        bass_nofuse=True,
                    )
                    insts.insert(pos, nop)
                    pos += 1
                    i += 1
                si.on_wait = keep
            i += 1


DEFAULT_CFG = dict(
    warm0=12,            # warmup dummy matmuls before the first projection
    warm_trickle=0,      # dummies interleaved after each slice-0 c-tile matmul
    lag=6,               # score groups the PV/outproj trail runs behind
    taper_mult=1,        # how aggressively the trail drains near the end
    gate_wqv=2,          # x-s0 chunk whose DMA gates the wq/wv SWDGE loads
    gate_wo=1,           # x slice whose last chunk gates the wo SWDGE load
    tail_act_copies=3,   # of the 6 tail out copies, how many go on Act
    tail_swdge=2,        # of the 6 tail stores, how many go on the gpsimd queue
    loop_n=1,
)


def _build_nc(cfg=None):
    import concourse.bass as bass
    import concourse.tile as tile
    import concourse.mybir as mybir
    from concourse.tile import add_dep_helper

    cfg = {**DEFAULT_CFG, **(cfg or {})}

    f32 = mybir.dt.float32
    f32r = mybir.dt.float32r
    bf16 = mybir.dt.bfloat16
    EXP = mybir.ActivationFunctionType.Exp
    COPY = mybir.ActivationFunctionType.Copy

    # weights arrive host-packed partition-major so every DMA descriptor is a
    # full 2KB+ contiguous row (small descriptors pay a 2x DMA penalty):
    #   wq/wk/wv: [128, ct*192+j] = W[ct*128+p, j]   (bf16)
    #   wo:       [96, h*768+c]  = W_out[h*96+p, c]  (f32)
    nc = bass.Bass(num_devices=N_CORES)
    x = nc.declare_dram_parameter("x", [C, S], bf16, isOutput=False)
    wqk = nc.declare_dram_parameter("wqk", [128, CT * 4 * D], bf16, isOutput=False)
    wv = nc.declare_dram_parameter("wv", [128, CT * 2 * D], bf16, isOutput=False)
    wo = nc.declare_dram_parameter("wo", [D, 2 * C], f32, isOutput=False)
    out = nc.declare_dram_parameter("out", [C, S], bf16, isOutput=True)

    with tile.TileContext(nc) as tc:
        with (
            tc.tile_pool(name="sb_x", bufs=1) as sb_x,
            tc.tile_pool(name="sb_w", bufs=1) as sb_w,
            tc.tile_pool(name="sb_qk", bufs=1) as sb_qk,
            tc.tile_pool(name="sb_v", bufs=1) as sb_v,
            tc.tile_pool(name="sb_p", bufs=cfg["lag"] + 2) as sb_p,
            tc.tile_pool(name="sb_o", bufs=3) as sb_o,
            tc.tile_pool(name="sb_m", bufs=6) as sb_m,
            tc.tile_pool(name="sb_oc", bufs=8) as sb_oc,
            tc.tile_pool(name="ps_proj", bufs=2, space="PSUM") as ps_proj,
            tc.tile_pool(name="ps_attn", bufs=2, space="PSUM") as ps_attn,
            tc.tile_pool(name="ps_o", bufs=2, space="PSUM") as ps_o,
        ):
          import contextlib
          loop_ctx = tc.For_i(0, cfg["loop_n"], 1) if cfg["loop_n"] > 1 else contextlib.nullcontext()
          with loop_ctx:
            # bf16 memset works directly (f32r doesn't), saving the
            # f32->f32r copy on the warmup critical path.
            # zr is a single partition row: the warmup matmul contracts K=1.
            zr = sb_w.tile([1, 512], bf16, name="zr")
            nc.vector.memset(zr[:], 0.0)
            cone = sb_w.tile([128, 96], f32, name="cone")
            nc.vector.memset(cone[:], 1.0)
            ones1 = sb_w.tile([1, D], f32r, name="ones1")
            nc.vector.tensor_copy(ones1[:], cone[0:1, :])

            def dummy_mm(n=1, w=512):
                """PE p-state warmers: zero-tile matmuls; w trades coverage
                per instruction against granularity (trickle uses w=64)."""
                for _ in range(n):
                    dps = ps_proj.tile([128, 512], f32, name="ps_proj")
                    nc.tensor.matmul(dps[0:64, 0:w], zr[:, 0:64], zr[:, 0:w],
                                     start=True, stop=True)

            # ---- input DMAs ----
            # slice 0 as per-(ct,512) chunks for fast availability; the rest
            # as two (128, 768) DMAs per c-tile (fewer HWDGE generations than
            # per-slice chunks, smoother arrival than one wide DMA)
            xt_s0 = {ct: sb_x.tile([128, 512], bf16, name=f"xt{ct}_0")
                     for ct in range(CT)}
            xt_rest = {ct: sb_x.tile([128, 3 * 512], bf16, name=f"xt{ct}_r")
                       for ct in range(CT)}
            x_dmas = {}

            def load_x_slice0():
                for ct in range(CT):
                    x_dmas[(ct, 0)] = nc.sync.dma_start(
                        xt_s0[ct][:],
                        x[ct * 128:(ct + 1) * 128, 0:512],
                    )

            def load_x_rest(ct, half):
                a, b = (0, 768) if half == 0 else (768, 1536)
                d = nc.sync.dma_start(
                    xt_rest[ct][:, a:b],
                    x[ct * 128:(ct + 1) * 128, 512 + a:512 + b],
                )
                # half 0 covers slice 1 and half of slice 2; half 1 the rest
                if half == 0:
                    x_dmas[(ct, 1)] = d
                else:
                    x_dmas[(ct, 2)] = d
                    x_dmas[(ct, 3)] = d

            class _XtView:
                def __init__(self, ct):
                    self.ct = ct
                def __getitem__(self, key):
                    rows, cols = key
                    a, b = cols.start or 0, cols.stop
                    assert b - a <= 512
                    if b <= 512:
                        return xt_s0[self.ct][rows, a:b]
                    assert a >= 512
                    return xt_rest[self.ct][rows, a - 512:b - 512]

            xt = [_XtView(ct) for ct in range(CT)]

            # weights: the packed q/k matrix on SWDGE immediately (its
            # transfer slots between the first x chunks); wv gated on a later
            # x-s0 chunk so slice 0 completes first; wo gated on x-rest.
            tqk = sb_w.tile([128, CT * 4 * D], bf16, name="wqk")
            nc.gpsimd.dma_start(tqk[:], wqk[:])
            wqk_t = [tqk[:, ct * 4 * D:(ct + 1) * 4 * D] for ct in range(CT)]

            load_x_slice0()

            tv = sb_w.tile([128, CT * 2 * D], bf16, name="wv")
            d_wv = nc.gpsimd.dma_start(tv[:], wv[:])
            gate = x_dmas[(cfg["gate_wqv"], 0)]
            add_dep_helper(d_wv.ins, gate.ins, sync=True, reason="wv after x s0")
            wv_t = [tv[:, ct * 2 * D:(ct + 1) * 2 * D] for ct in range(CT)]

            for ct in range(CT):
                load_x_rest(ct, 0)
            for ct in range(CT):
                load_x_rest(ct, 1)

            two = sb_w.tile([D, 2 * C], f32r, name="wo")
            d_wo = nc.gpsimd.dma_start(two[:], wo[:].bitcast(f32r))
            add_dep_helper(d_wo.ins, x_dmas[(CT - 1, cfg["gate_wo"])].ins,
                           sync=True, reason="wo after x")
            wo_t = [two[:, h * C:(h + 1) * C] for h in range(2)]

            # ---- persistent compute tiles ----
            # The packed projection yields 3 full 128-row tiles per slice:
            #   stg0 rows 0-95 = q0, 96-127 = q1 d0-31
            #   stg1 rows 0-63 = q1 d32-95, 64-127 = k0 d0-63
            #   stg2 rows 0-31 = k0 d64-95, 32-127 = k1
            # q0 is used in place (stg0 view); q1/k0/k1 are reassembled by
            # partition-shifted gpsimd copies on the otherwise idle pool.
            stg = [sb_qk.tile([128, S], f32r, name=f"stg{t}") for t in range(3)]
            # h1 runs a K=128 contraction: kT1 is stg2 in place (k1 at rows
            # 32-127, k0 spill at rows 0-31) and qT1 holds q1 at rows 32-127
            # with rows 0-31 zeroed, so the mismatched rows contribute
            # finite * 0 = 0.
            qT1full = sb_qk.tile([128, S], f32r, name="qT1")
            zq = sb_w.tile([32, S], f32, name="zq")
            nc.vector.memset(zq[:], 0.0)
            nc.vector.tensor_copy(qT1full[0:32, :], zq[:])
            qT = [stg[0][0:D, :], qT1full[:]]
            kT = [sb_qk.tile([D, S], f32r, name="kT0"), stg[2][:]]
            v_cat = [sb_v.tile([128, JT, D + 1], f32r, name=f"v{h}") for h in range(2)]
            for h in range(2):
                nc.vector.tensor_copy(v_cat[h][:, :, D], cone[:, 0:JT])

            def proj_qk3(s, trickle=0):
                sl = slice(s * 512, (s + 1) * 512)
                for t in range(3):
                    acc = ps_proj.tile([128, 512], f32, name="ps_proj")
                    for ct in range(CT):
                        nc.tensor.matmul(
                            acc[:],
                            wqk_t[ct][:, t * 128:(t + 1) * 128],
                            xt[ct][:, s * 512:(s + 1) * 512],
                            start=(ct == 0), stop=(ct == CT - 1),
                        )
                        if trickle and t == 0:
                            dummy_mm(trickle, w=64)
                    nc.vector.tensor_copy(stg[t][:, sl], acc[:])
                # partition-shifted reassembly in 32-partition pieces (the
                # compiler limits partition windows to 32 from a 32-aligned
                # start); only k here — the attention lead needs kT; qT1 for
                # slice s isn't consumed until the lead reaches isl s, so its
                # copies are deferred off the pool's critical path
                nc.gpsimd.tensor_copy(kT[0][0:64, sl], stg[1][64:128, sl])
                nc.gpsimd.tensor_copy(kT[0][64:D, sl], stg[2][0:32, sl])

            def emit_q_copies(s):
                # q1 into rows 32-127 of qT1full (matching kT1's d -> d+32)
                sl = slice(s * 512, (s + 1) * 512)
                nc.gpsimd.tensor_copy(qT1full[32:64, sl], stg[0][D:128, sl])
                nc.gpsimd.tensor_copy(qT1full[64:128, sl], stg[1][0:64, sl])

            def proj_v(jt):
                accv = ps_proj.tile([128, 512], f32, name="ps_proj")
                for ct in range(CT):
                    nc.tensor.matmul(
                        accv[:, 0:2 * D],
                        xt[ct][:, jt * 128:(jt + 1) * 128],
                        wv_t[ct][:],
                        start=(ct == 0), stop=(ct == CT - 1),
                    )
                for h in range(2):
                    nc.vector.tensor_copy(v_cat[h][:, jt, 0:D], accv[:, h * D:(h + 1) * D])

            # ---- attention machinery ----
            # score groups: (h, isl, g) covers key tiles jt in {2g, 2g+1}
            sg_tiles = {}
            exp_tiles = {}

            def emit_scores(h, isl, g):
                sg = ps_attn.tile([128, 1024], f32, name="ps_attn")
                for t, jt in enumerate((2 * g, 2 * g + 1)):
                    nc.tensor.matmul(
                        sg[:, t * 512:(t + 1) * 512],
                        kT[h][:, jt * 128:(jt + 1) * 128],
                        qT[h][:, isl * 512:(isl + 1) * 512],
                        start=True, stop=True,
                    )
                pt = sb_p.tile([128, 1024], f32r, name="pt")
                nc.scalar.activation(pt[:], sg[:], EXP)
                sg_tiles[(h, isl, g)] = sg
                exp_tiles[(h, isl, g)] = pt

            oacc = {}

            def emit_pv(h, isl, g):
                if g == 0:
                    oacc[(h, isl)] = ps_o.tile([D + 1, 512], f32, name="ps_o")
                pt = exp_tiles.pop((h, isl, g))
                del sg_tiles[(h, isl, g)]
                Oacc = oacc[(h, isl)]
                for t, jt in enumerate((2 * g, 2 * g + 1)):
                    nc.tensor.matmul(
                        Oacc[:],
                        v_cat[h][:, jt, :],
                        pt[:, t * 512:(t + 1) * 512],
                        start=(jt == 0), stop=(jt == JT - 1),
                    )

            def emit_recip(h, isl):
                Oacc = oacc[(h, isl)]
                recip_r = sb_m.tile([1, 512], f32r, name="recip_r")
                with nc.allow_low_precision("softmax denominator reciprocal"):
                    nc.vector.reciprocal(recip_r[:], Oacc[D:D + 1, :])
                return recip_r

            def emit_bc(recip_r):
                bc_ps = ps_proj.tile([128, 512], f32, name="ps_proj")
                nc.tensor.matmul(bc_ps[0:D, :], ones1[:], recip_r[:],
                                 start=True, stop=True)
                return bc_ps

            def emit_mul(h, isl, bc_ps, copy_eng=None):
                # the DVE can read only one PSUM operand: copy the
                # unnormalized Oacc to SBUF (in parallel with the reciprocal /
                # bc broadcast), then multiply SBUF x PSUM. Also frees the
                # Oacc bank early.
                Oacc = oacc.pop((h, isl))
                ou = sb_m.tile([D, 512], f32, name="ou")
                if copy_eng is nc.scalar:
                    nc.scalar.activation(ou[:], Oacc[0:D, :], COPY)
                else:
                    nc.vector.tensor_copy(ou[:], Oacc[0:D, :])
                o = sb_o.tile([D, 512], f32r, name="o_n")
                nc.vector.tensor_mul(o[:], ou[:], bc_ps[0:D, :])
                return o

            def emit_outproj_ct(isl, ct, o0, o1, tail_i=None):
                po = ps_proj.tile([128, 512], f32, name="ps_proj")
                for h, o in ((0, o0), (1, o1)):
                    nc.tensor.matmul(
                        po[:],
                        wo_t[h][:, ct * 128:(ct + 1) * 128],
                        o[:],
                        start=(h == 0), stop=(h == 1),
                    )
                oc = sb_oc.tile([128, 512], bf16, name="oc")
                if tail_i is not None and tail_i < cfg["tail_act_copies"]:
                    nc.scalar.activation(oc[:], po[:], COPY)
                else:
                    nc.vector.tensor_copy(oc[:], po[:])
                dst = out[ct * 128:(ct + 1) * 128, isl * 512:(isl + 1) * 512]
                if tail_i is not None and tail_i < cfg["tail_swdge"]:
                    nc.gpsimd.dma_start(dst, oc[:])
                else:
                    nc.sync.dma_start(dst, oc[:])

            # ---- decoupled lead/trail emission ----
            # The lead stream (scores + exp) runs as early as possible so the
            # scalar engine — whose 68us of exp work would otherwise gate the
            # kernel tail — finishes mid-kernel. The trail stream (PV + norm +
            # out-projection + stores) lags `lag` score groups behind, living
            # off the deep sb_p pt pool; it doubles as the PE filler that
            # paces the lead to the scalar engine's throughput.
            import collections
            trail_q = collections.deque()
            lt_state = {"lead": 0, "trail": 0}
            o_norm = {}

            def pump_trail(target):
                while trail_q and lt_state["trail"] < target:
                    kind, fn = trail_q.popleft()
                    fn()
                    if kind == "pv":
                        lt_state["trail"] += 1

            def trail_norm(isl):
                r0 = emit_recip(0, isl)
                r1 = emit_recip(1, isl)
                bc0 = emit_bc(r0)
                bc1 = emit_bc(r1)
                o_norm[isl] = (emit_mul(0, isl, bc0), emit_mul(1, isl, bc1))

            def trail_outproj(isl, cts):
                o0, o1 = o_norm[isl]
                for ct in cts:
                    emit_outproj_ct(isl, ct, o0, o1)

            def emit_lead(isl, g):
                emit_scores(0, isl, g)
                emit_scores(1, isl, g)
                trail_q.append(("pv", lambda isl=isl, g=g: (
                    emit_pv(0, isl, g), emit_pv(1, isl, g))))
                if g == JG - 1 and isl < IT - 1:
                    trail_q.append(("aux", lambda isl=isl: trail_norm(isl)))
                    for cts in ((0, 1), (2, 3), (4, 5)):
                        trail_q.append(
                            ("aux", lambda isl=isl, cts=cts: trail_outproj(isl, cts)))
                lt_state["lead"] += 1
                # taper: near the end of the lead stream, drain the trail
                # deeper so the Act-paced score stalls are filled with PV work
                # and little trail remains after the last scores
                n_lead_total = IT * JG
                taper = max(0, lt_state["lead"] - (n_lead_total - cfg["lag"] + 2))
                pump_trail(lt_state["lead"] - cfg["lag"] + cfg["taper_mult"] * taper)

            # phase 1: slice-pipelined projections, isl0's lead as each key
            # slice lands
            for s in range(IT):
                trickle = cfg["warm_trickle"] if s == 0 else 0
                if s == 0:
                    dummy_mm(cfg["warm0"])
                proj_qk3(s, trickle=trickle)
                if s == 0:
                    emit_q_copies(0)
                for jt in range(4 * s, 4 * s + 4):
                    proj_v(jt)
                if s > 0:
                    emit_q_copies(s)
                for g in range(2 * s, 2 * s + 2):
                    emit_lead(0, g)

            # phase 2: remaining slices' lead, trail pumping throughout
            for isl in range(1, IT):
                for g in range(JG):
                    emit_lead(isl, g)
            pump_trail(10 ** 9)

            # ---- tail: isl3 normalize + out-projection + stores ----
            # h0's half of the out-projection starts as soon as o0 is ready
            # (po tiles: 2 from ps_proj + 4 carved from the now-free ps_attn
            # tiles); h1 accumulates into them once o1 lands. Copies alternate
            # DVE/Act per chunk; the earliest stores ride the SWDGE lane.
            isl = IT - 1
            po = [None] * CT

            def mm_out(h, ct, o, stop):
                nc.tensor.matmul(
                    po[ct][:], wo_t[h][:, ct * 128:(ct + 1) * 128], o[:],
                    start=(h == 0), stop=stop,
                )

            # tail pipeline (isl3's PVs already ran in the trail): h0's exp
            # and PV finish first, so h0's normalize + opening out-proj
            # matmuls overlap h1's final exp + PV; h1 closes the accumulation
            # with copy + store chasing each closing matmul.
            r0 = emit_recip(0, isl)
            bc0 = emit_bc(r0)
            o0 = emit_mul(0, isl, bc0, copy_eng=nc.scalar)
            r1 = emit_recip(1, isl)
            CT_ORDER = (2, 3, 4, 5, 0, 1)
            for ct in CT_ORDER[:4]:
                if ct % 2 == 0:
                    big = ps_attn.tile([128, 1024], f32, name="ps_attn")
                    po[ct] = big[:, 0:512]
                else:
                    po[ct] = big[:, 512:1024]
                nc.tensor.matmul(
                    po[ct][:], wo_t[0][:, ct * 128:(ct + 1) * 128], o0[:],
                    start=True, stop=False,
                )
            bc1 = emit_bc(r1)
            o1 = emit_mul(1, isl, bc1, copy_eng=nc.scalar)
            for ct in CT_ORDER[4:]:
                po[ct] = ps_o.tile([128, 512], f32, name="ps_o")
                nc.tensor.matmul(
                    po[ct][:], wo_t[0][:, ct * 128:(ct + 1) * 128], o0[:],
                    start=True, stop=False,
                )
            for i, ct in enumerate(CT_ORDER):
                nc.tensor.matmul(
                    po[ct][:], wo_t[1][:, ct * 128:(ct + 1) * 128], o1[:],
                    start=False, stop=True,
                )
                oc = sb_oc.tile([128, 512], bf16, name="oc")
                if i % 2 == 0:
                    nc.scalar.activation(oc[:], po[ct][:], COPY)
                else:
                    nc.vector.tensor_copy(oc[:], po[ct][:])
                dst = out[ct * 128:(ct + 1) * 128, isl * 512:(isl + 1) * 512]
                if i < cfg["tail_swdge"]:
                    nc.gpsimd.dma_start(dst, oc[:])
                else:
                    nc.sync.dma_start(dst, oc[:])

    _split_sync_waits(nc, mybir)
    return nc


class _Runner:
    """Compile once, run many. Mirrors run_bass_via_pjrt's multi-core path but
    keeps the jitted executable cached across calls."""

    def __init__(self, cfg=None):
        import jax
        import concourse.mybir as mybir
        from concourse import bass2jax
        from jax.sharding import Mesh, PartitionSpec
        from jax.experimental.shard_map import shard_map

        self.jax = jax
        nc = _build_nc(cfg)
        self.nc = nc
        bass2jax.install_neuronx_cc_hook()

        in_names, out_names, out_avals = [], [], []
        for alloc in nc.m.functions[0].allocations:
            if not isinstance(alloc, mybir.MemoryLocationSet):
                continue
            name = alloc.memorylocations[0].name
            if alloc.kind == "ExternalInput":
                if nc.partition_id_tensor is None or name != nc.partition_id_tensor.name:
                    in_names.append(name)
            elif alloc.kind == "ExternalOutput":
                out_names.append(name)
                out_avals.append(
                    jax.core.ShapedArray(tuple(alloc.tensor_shape), mybir.dt.np(alloc.dtype))
                )
        self.in_names = in_names
        self.out_names = out_names
        partition_name = nc.partition_id_tensor.name if nc.partition_id_tensor else None
        all_names = tuple(in_names + out_names + ([partition_name] if partition_name else []))

        def _body(*args):
            operands = list(args)
            if partition_name is not None:
                operands.append(bass2jax.partition_id_tensor())
            outs = bass2jax._bass_exec_p.bind(
                *operands,
                out_avals=tuple(out_avals),
                in_names=all_names,
                out_names=tuple(out_names),
                lowering_input_output_aliases=(),
                sim_require_finite=True,
                sim_require_nnan=True,
                nc=nc,
            )
            return tuple(outs)

        devices = jax.devices()[:N_CORES]
        mesh = Mesh(np.asarray(devices), ("core",))
        n_all = len(in_names) + len(out_names)
        self.sharded = jax.jit(
            shard_map(
                _body,
                mesh=mesh,
                in_specs=(PartitionSpec("core"),) * n_all,
                out_specs=(PartitionSpec("core"),) * len(out_names),
                check_rep=False,
            ),
            keep_unused=True,
        )
        self.out_shapes = [tuple(a.shape) for a in out_avals]
        self.out_dtypes = [a.dtype for a in out_avals]

    def run(self, in_maps):
        concat_in = [
            np.concatenate([np.asarray(in_maps[c][n]) for c in range(N_CORES)], axis=0)
            for n in self.in_names
        ]
        concat_zero = [
            np.zeros((N_CORES * s[0], *s[1:]), d)
            for s, d in zip(self.out_shapes, self.out_dtypes)
        ]
        outs = self.sharded(*concat_in, *concat_zero)
        self.jax.block_until_ready(outs)
        return [
            {
                n: np.asarray(outs[i]).reshape(N_CORES, *self.out_shapes[i])[c]
                for i, n in enumerate(self.out_names)
            }
            for c in range(N_CORES)
        ]


def _get_runner():
    global _RUNNER
    if _RUNNER is None:
        _RUNNER = _Runner()
    return _RUNNER


def _pack_w(w):
    """(768, 192) -> (128, 6*192) partition-major: out[p, ct*192+j] = w[ct*128+p, j]."""
    return np.ascontiguousarray(
        w.reshape(CT, 128, 2 * D).transpose(1, 0, 2).reshape(128, CT * 2 * D)
    )


def _shard_inputs(inputs, W_qkv, W_out):
    import ml_dtypes

    bf16 = ml_dtypes.bfloat16
    in_maps = []
    for core in range(N_CORES):
        b, g = divmod(core, 4)
        cols = slice(g * 2 * D, (g + 1) * 2 * D)
        wo = W_out[cols, :]  # (192, 768)
        wo_packed = np.ascontiguousarray(
            wo.reshape(2, D, C).transpose(1, 0, 2).reshape(D, 2 * C)
        )
        # packed q/k: per c-tile the 384 output dims are [q0|q1|k0|k1]
        q = W_qkv[:, cols].reshape(CT, 128, 2 * D)
        k = W_qkv[:, 768:][:, cols].reshape(CT, 128, 2 * D)
        wqk = np.concatenate([q, k], axis=2)  # (CT, 128, 384)
        wqk = np.ascontiguousarray(
            wqk.transpose(1, 0, 2).reshape(128, CT * 4 * D)
        )
        in_maps.append({
            "x": np.ascontiguousarray(inputs[b]).astype(bf16),
            "wqk": wqk.astype(bf16),
            "wv": _pack_w(W_qkv[:, 1536:][:, cols]).astype(bf16),
            "wo": wo_packed,
        })
    return in_maps


def kernel(inputs, W_qkv, W_out):
    inputs = np.asarray(inputs, dtype=np.float32)
    W_qkv = np.asarray(W_qkv, dtype=np.float32)
    W_out = np.asarray(W_out, dtype=np.float32)
    runner = _get_runner()
    results = runner.run(_shard_inputs(inputs, W_qkv, W_out))
    out = np.zeros((B, C, S), np.float32)
    for core in range(N_CORES):
        out[core // 4] += results[core]["out"].astype(np.float32)
    return out


# revision 68
# speedup vs baseline: 1.2254x; 1.0277x over previous
"""Multi-head attention (b=2, c=768, s=2048, 8 heads, d=96) on 8 TRN2 NeuronCores.

Sharding: batch x head-group tensor parallel. Core i handles batch i//4 and
heads {2*(i%4), 2*(i%4)+1}; the host sums the 4 partial outputs per batch
element (the all-reduce of the sharding hint, done host-side since the kernel
returns full outputs anyway).

v2 schedule (104.3us TimelineSim vs the v1 baseline's 127.9us; measured HW
rel err ~9e-3 against the f32 reference, tolerance 2e-2):
  - x and the QKV weights are uploaded as bf16 (host-converted, host-packed
    partition-major so every DMA descriptor is a full 2KB+ row): halves the
    input DMA stream and lets the v-projection run at N=192 without the f32r
    N>=256 zero-padding. Scores / PV / out-projection stay f32r.
  - the q/k weights pack as [q0|q1|k0|k1] per c-tile so the projection runs
    as 3 full 128-row matmul tiles per slice instead of 4 x 96-row ones
    (18 vs 24 matmuls, -5us PE). q0 is consumed in place; k0 is reassembled
    by two partition-shifted gpsimd copies; h1 contracts over K=128 with
    kT1 read in place from the staging tile (k1 at rows 32-127) against a
    q1 tile whose rows 0-31 are zeroed, so the spill rows contribute 0.
  - output stores are bf16 (host upcasts and sums the per-core partials):
    halves store traffic so the final-slice store burst shrinks.
  - PE warmup: dummy matmuls on a zero tile keep the tensor engine's p-state
    ramp warm while the first x chunks stream in; slice 0 loads as 512-wide
    chunks, the rest as 768-wide chunks (HWDGE generation, 625ns per DMA, is
    the input-stream limiter once transfers are bf16).
  - attention is emitted as a decoupled lead stream (scores + exp, as early
    as each key slice's projections land — exp starts ~8us in) and a trail
    stream (PV + normalize + out-projection + stores) that lags `lag` score
    groups behind, buffered in the deep sb_p pt pool. The trail doubles as
    the PE filler that paces the lead to the scalar engine's exp throughput
    (the attention-phase bottleneck at ~66us busy), so the scalar engine
    finishes its exp queue mid-kernel instead of gating the tail.
  - normalize: the softmax denominator row (PV's ones-column) is
    reciprocal'd on DVE, broadcast across partitions via a K=1 matmul, and
    multiplied against an SBUF copy of the unnormalized Oacc (the DVE can
    read only one PSUM operand per instruction).
  - tail: h0's normalize + opening out-proj matmuls overlap h1's final
    exp/PV; po tiles borrow the freed ps_attn banks; copies alternate
    Act/DVE per chunk and the first stores ride the gpsimd SWDGE lane so
    descriptor generation runs in two lanes.
"""

import numpy as np

N_CORES = 8
B, C, S = 2, 768, 2048
H, D = 8, 96
CT = C // 128          # 6 c-tiles
IT = S // 512          # 4 query slices
JT = S // 128          # 16 key tiles
JG = JT // 2           # 8 exp groups of 2 key tiles

_RUNNER = None


def _split_sync_waits(nc, mybir, max_waits=1):
    """This walrus build rejects instructions carrying more than one sem wait
    (setupSyncWait: 'Too many sync wait commands'). Split excess waits onto
    same-engine NoOps inserted immediately before the instruction."""
    for bb in nc.main_func.blocks:
        insts = bb.instructions
        i = 0
        while i < len(insts):
            inst = insts[i]
            si = inst.sync_info
            if si is not None and si.on_wait and len(si.on_wait) > max_waits:
                waits = list(si.on_wait)
                keep = waits[-max_waits:]
                extra = waits[:-max_waits]
                pos = i
                while extra:
                    chunk, extra = extra[:max_waits], extra[max_waits:]
                    nop = mybir.InstNoOp(
                        name=nc.get_next_instruction_name(),
                        sync_info=mybir.SyncInfo(on_wait=chunk, on_update=[]),
                        engine=inst.engine,
                        bass_nofuse=True,
                    )
                    insts.insert(pos, nop)
                    pos += 1
                    i += 1
                si.on_wait = keep
            i += 1


DEFAULT_CFG = dict(
    warm0=10,            # warmup dummy matmuls before the first projection
    warm_trickle=0,      # dummies interleaved after each slice-0 c-tile matmul
    lag=6,               # score groups the PV/outproj trail runs behind
    taper_mult=1,        # how aggressively the trail drains near the end
    gate_wqv=2,          # x-s0 chunk whose DMA gates the wq/wv SWDGE loads
    gate_wo=1,           # x slice whose last chunk gates the wo SWDGE load
    tail_act_copies=3,   # of the 6 tail out copies, how many go on Act
    tail_swdge=2,        # of the 6 tail stores, how many go on the gpsimd queue
    loop_n=1,
)


def _build_nc(cfg=None):
    import concourse.bass as bass
    import concourse.tile as tile
    import concourse.mybir as mybir
    from concourse.tile import add_dep_helper

    cfg = {**DEFAULT_CFG, **(cfg or {})}

    f32 = mybir.dt.float32
    f32r = mybir.dt.float32r
    bf16 = mybir.dt.bfloat16
    EXP = mybir.ActivationFunctionType.Exp
    COPY = mybir.ActivationFunctionType.Copy

    # weights arrive host-packed partition-major so every DMA descriptor is a
    # full 2KB+ contiguous row (small descriptors pay a 2x DMA penalty):
    #   wq/wk/wv: [128, ct*192+j] = W[ct*128+p, j]   (bf16)
    #   wo:       [96, h*768+c]  = W_out[h*96+p, c]  (f32)
    nc = bass.Bass(num_devices=N_CORES)
    x = nc.declare_dram_parameter("x", [C, S], bf16, isOutput=False)
    wqk = nc.declare_dram_parameter("wqk", [128, CT * 4 * D], bf16, isOutput=False)
    wv = nc.declare_dram_parameter("wv", [128, CT * 2 * D], bf16, isOutput=False)
    wo = nc.declare_dram_parameter("wo", [D, 2 * C], f32, isOutput=False)
    out = nc.declare_dram_parameter("out", [C, S], bf16, isOutput=True)

    with tile.TileContext(nc) as tc:
        with (
            tc.tile_pool(name="sb_x", bufs=1) as sb_x,
            tc.tile_pool(name="sb_w", bufs=1) as sb_w,
            tc.tile_pool(name="sb_qk", bufs=1) as sb_qk,
            tc.tile_pool(name="sb_v", bufs=1) as sb_v,
            tc.tile_pool(name="sb_p", bufs=cfg["lag"] + 2) as sb_p,
            tc.tile_pool(name="sb_o", bufs=3) as sb_o,
            tc.tile_pool(name="sb_m", bufs=6) as sb_m,
            tc.tile_pool(name="sb_oc", bufs=8) as sb_oc,
            tc.tile_pool(name="ps_proj", bufs=2, space="PSUM") as ps_proj,
            tc.tile_pool(name="ps_attn", bufs=2, space="PSUM") as ps_attn,
            tc.tile_pool(name="ps_o", bufs=2, space="PSUM") as ps_o,
        ):
          import contextlib
          loop_ctx = tc.For_i(0, cfg["loop_n"], 1) if cfg["loop_n"] > 1 else contextlib.nullcontext()
          with loop_ctx:
            # bf16 memset works directly (f32r doesn't), saving the
            # f32->f32r copy on the warmup critical path.
            # zr is a single partition row: the warmup matmul contracts K=1.
            zr = sb_w.tile([1, 512], bf16, name="zr")
            nc.vector.memset(zr[:], 0.0)
            cone = sb_w.tile([128, 96], f32, name="cone")
            nc.vector.memset(cone[:], 1.0)
            ones1 = sb_w.tile([1, D], f32r, name="ones1")
            nc.vector.tensor_copy(ones1[:], cone[0:1, :])

            def dummy_mm(n=1, w=512):
                """PE p-state warmers: zero-tile matmuls; w trades coverage
                per instruction against granularity (trickle uses w=64)."""
                for _ in range(n):
                    dps = ps_proj.tile([128, 512], f32, name="ps_proj")
                    nc.tensor.matmul(dps[0:64, 0:w], zr[:, 0:64], zr[:, 0:w],
                                     start=True, stop=True)

            # ---- input DMAs ----
            # slice 0 as per-(ct,512) chunks for fast availability; the rest
            # as two (128, 768) DMAs per c-tile (fewer HWDGE generations than
            # per-slice chunks, smoother arrival than one wide DMA)
            xt_s0 = {ct: sb_x.tile([128, 512], bf16, name=f"xt{ct}_0")
                     for ct in range(CT)}
            xt_rest = {ct: sb_x.tile([128, 3 * 512], bf16, name=f"xt{ct}_r")
                       for ct in range(CT)}
            x_dmas = {}

            def load_x_slice0():
                for ct in range(CT):
                    x_dmas[(ct, 0)] = nc.sync.dma_start(
                        xt_s0[ct][:],
                        x[ct * 128:(ct + 1) * 128, 0:512],
                    )

            def load_x_rest(ct, half):
                a, b = (0, 768) if half == 0 else (768, 1536)
                d = nc.sync.dma_start(
                    xt_rest[ct][:, a:b],
                    x[ct * 128:(ct + 1) * 128, 512 + a:512 + b],
                )
                # half 0 covers slice 1 and half of slice 2; half 1 the rest
                if half == 0:
                    x_dmas[(ct, 1)] = d
                else:
                    x_dmas[(ct, 2)] = d
                    x_dmas[(ct, 3)] = d

            class _XtView:
                def __init__(self, ct):
                    self.ct = ct
                def __getitem__(self, key):
                    rows, cols = key
                    a, b = cols.start or 0, cols.stop
                    assert b - a <= 512
                    if b <= 512:
                        return xt_s0[self.ct][rows, a:b]
                    assert a >= 512
                    return xt_rest[self.ct][rows, a - 512:b - 512]

            xt = [_XtView(ct) for ct in range(CT)]

            # weights: the packed q/k matrix on SWDGE immediately (its
            # transfer slots between the first x chunks); wv gated on a later
            # x-s0 chunk so slice 0 completes first; wo gated on x-rest.
            tqk = sb_w.tile([128, CT * 4 * D], bf16, name="wqk")
            nc.gpsimd.dma_start(tqk[:], wqk[:])
            wqk_t = [tqk[:, ct * 4 * D:(ct + 1) * 4 * D] for ct in range(CT)]

            load_x_slice0()

            tv = sb_w.tile([128, CT * 2 * D], bf16, name="wv")
            d_wv = nc.gpsimd.dma_start(tv[:], wv[:])
            gate = x_dmas[(cfg["gate_wqv"], 0)]
            add_dep_helper(d_wv.ins, gate.ins, sync=True, reason="wv after x s0")
            wv_t = [tv[:, ct * 2 * D:(ct + 1) * 2 * D] for ct in range(CT)]

            for ct in range(CT):
                load_x_rest(ct, 0)
            for ct in range(CT):
                load_x_rest(ct, 1)

            two = sb_w.tile([D, 2 * C], f32r, name="wo")
            d_wo = nc.gpsimd.dma_start(two[:], wo[:].bitcast(f32r))
            add_dep_helper(d_wo.ins, x_dmas[(CT - 1, cfg["gate_wo"])].ins,
                           sync=True, reason="wo after x")
            wo_t = [two[:, h * C:(h + 1) * C] for h in range(2)]

            # ---- persistent compute tiles ----
            # The packed projection yields 3 full 128-row tiles per slice:
            #   stg0 rows 0-95 = q0, 96-127 = q1 d0-31
            #   stg1 rows 0-63 = q1 d32-95, 64-127 = k0 d0-63
            #   stg2 rows 0-31 = k0 d64-95, 32-127 = k1
            # q0 is used in place (stg0 view); q1/k0/k1 are reassembled by
            # partition-shifted gpsimd copies on the otherwise idle pool.
            stg = [sb_qk.tile([128, S], f32r, name=f"stg{t}") for t in range(3)]
            # h1 runs a K=128 contraction: kT1 is stg2 in place (k1 at rows
            # 32-127, k0 spill at rows 0-31) and qT1 holds q1 at rows 32-127
            # with rows 0-31 zeroed, so the mismatched rows contribute
            # finite * 0 = 0.
            qT1full = sb_qk.tile([128, S], f32r, name="qT1")
            zq = sb_w.tile([32, S], f32, name="zq")
            nc.vector.memset(zq[:], 0.0)
            nc.vector.tensor_copy(qT1full[0:32, :], zq[:])
            qT = [stg[0][0:D, :], qT1full[:]]
            kT = [sb_qk.tile([D, S], f32r, name="kT0"), stg[2][:]]
            v_cat = [sb_v.tile([128, JT, D + 1], f32r, name=f"v{h}") for h in range(2)]
            for h in range(2):
                nc.vector.tensor_copy(v_cat[h][:, :, D], cone[:, 0:JT])

            def proj_qk3(s, trickle=0):
                sl = slice(s * 512, (s + 1) * 512)
                for t in range(3):
                    acc = ps_proj.tile([128, 512], f32, name="ps_proj")
                    for ct in range(CT):
                        nc.tensor.matmul(
                            acc[:],
                            wqk_t[ct][:, t * 128:(t + 1) * 128],
                            xt[ct][:, s * 512:(s + 1) * 512],
                            start=(ct == 0), stop=(ct == CT - 1),
                        )
                        if trickle and t == 0:
                            dummy_mm(trickle, w=64)
                    nc.vector.tensor_copy(stg[t][:, sl], acc[:])
                # partition-shifted reassembly in 32-partition pieces (the
                # compiler limits partition windows to 32 from a 32-aligned
                # start); only k here — the attention lead needs kT; qT1 for
                # slice s isn't consumed until the lead reaches isl s, so its
                # copies are deferred off the pool's critical path
                nc.gpsimd.tensor_copy(kT[0][0:64, sl], stg[1][64:128, sl])
                nc.gpsimd.tensor_copy(kT[0][64:D, sl], stg[2][0:32, sl])

            def emit_q_copies(s):
                # q1 into rows 32-127 of qT1full (matching kT1's d -> d+32)
                sl = slice(s * 512, (s + 1) * 512)
                nc.gpsimd.tensor_copy(qT1full[32:64, sl], stg[0][D:128, sl])
                nc.gpsimd.tensor_copy(qT1full[64:128, sl], stg[1][0:64, sl])

            def proj_v(jt):
                accv = ps_proj.tile([128, 512], f32, name="ps_proj")
                for ct in range(CT):
                    nc.tensor.matmul(
                        accv[:, 0:2 * D],
                        xt[ct][:, jt * 128:(jt + 1) * 128],
                        wv_t[ct][:],
                        start=(ct == 0), stop=(ct == CT - 1),
                    )
                for h in range(2):
                    nc.vector.tensor_copy(v_cat[h][:, jt, 0:D], accv[:, h * D:(h + 1) * D])

            # ---- attention machinery ----
            # score groups: (h, isl, g) covers key tiles jt in {2g, 2g+1}
            sg_tiles = {}
            exp_tiles = {}

            def emit_scores(h, isl, g):
                sg = ps_attn.tile([128, 1024], f32, name="ps_attn")
                for t, jt in enumerate((2 * g, 2 * g + 1)):
                    nc.tensor.matmul(
                        sg[:, t * 512:(t + 1) * 512],
                        kT[h][:, jt * 128:(jt + 1) * 128],
                        qT[h][:, isl * 512:(isl + 1) * 512],
                        start=True, stop=True,
                    )
                pt = sb_p.tile([128, 1024], f32r, name="pt")
                nc.scalar.activation(pt[:], sg[:], EXP)
                sg_tiles[(h, isl, g)] = sg
                exp_tiles[(h, isl, g)] = pt

            oacc = {}

            def emit_pv(h, isl, g):
                if g == 0:
                    oacc[(h, isl)] = ps_o.tile([D + 1, 512], f32, name="ps_o")
                pt = exp_tiles.pop((h, isl, g))
                del sg_tiles[(h, isl, g)]
                Oacc = oacc[(h, isl)]
                for t, jt in enumerate((2 * g, 2 * g + 1)):
                    nc.tensor.matmul(
                        Oacc[:],
                        v_cat[h][:, jt, :],
                        pt[:, t * 512:(t + 1) * 512],
                        start=(jt == 0), stop=(jt == JT - 1),
                    )

            def emit_recip(h, isl):
                Oacc = oacc[(h, isl)]
                recip_r = sb_m.tile([1, 512], f32r, name="recip_r")
                with nc.allow_low_precision("softmax denominator reciprocal"):
                    nc.vector.reciprocal(recip_r[:], Oacc[D:D + 1, :])
                return recip_r

            def emit_bc(recip_r):
                bc_ps = ps_proj.tile([128, 512], f32, name="ps_proj")
                nc.tensor.matmul(bc_ps[0:D, :], ones1[:], recip_r[:],
                                 start=True, stop=True)
                return bc_ps

            def emit_mul(h, isl, bc_ps, copy_eng=None):
                # the DVE can read only one PSUM operand: copy the
                # unnormalized Oacc to SBUF (in parallel with the reciprocal /
                # bc broadcast), then multiply SBUF x PSUM. Also frees the
                # Oacc bank early.
                Oacc = oacc.pop((h, isl))
                ou = sb_m.tile([D, 512], f32, name="ou")
                if copy_eng is nc.scalar:
                    nc.scalar.activation(ou[:], Oacc[0:D, :], COPY)
                else:
                    nc.vector.tensor_copy(ou[:], Oacc[0:D, :])
                o = sb_o.tile([D, 512], f32r, name="o_n")
                nc.vector.tensor_mul(o[:], ou[:], bc_ps[0:D, :])
                return o

            def emit_outproj_ct(isl, ct, o0, o1, tail_i=None):
                po = ps_proj.tile([128, 512], f32, name="ps_proj")
                for h, o in ((0, o0), (1, o1)):
                    nc.tensor.matmul(
                        po[:],
                        wo_t[h][:, ct * 128:(ct + 1) * 128],
                        o[:],
                        start=(h == 0), stop=(h == 1),
                    )
                oc = sb_oc.tile([128, 512], bf16, name="oc")
                if tail_i is not None and tail_i < cfg["tail_act_copies"]:
                    nc.scalar.activation(oc[:], po[:], COPY)
                else:
                    nc.vector.tensor_copy(oc[:], po[:])
                dst = out[ct * 128:(ct + 1) * 128, isl * 512:(isl + 1) * 512]
                if tail_i is not None and tail_i < cfg["tail_swdge"]:
                    nc.gpsimd.dma_start(dst, oc[:])
                else:
                    nc.sync.dma_start(dst, oc[:])

            # ---- decoupled lead/trail emission ----
            # The lead stream (scores + exp) runs as early as possible so the
            # scalar engine — whose 68us of exp work would otherwise gate the
            # kernel tail — finishes mid-kernel. The trail stream (PV + norm +
            # out-projection + stores) lags `lag` score groups behind, living
            # off the deep sb_p pt pool; it doubles as the PE filler that
            # paces the lead to the scalar engine's throughput.
            import collections
            trail_q = collections.deque()
            lt_state = {"lead": 0, "trail": 0}
            o_norm = {}

            def pump_trail(target):
                while trail_q and lt_state["trail"] < target:
                    kind, fn = trail_q.popleft()
                    fn()
                    if kind == "pv":
                        lt_state["trail"] += 1

            def trail_norm(isl):
                r0 = emit_recip(0, isl)
                r1 = emit_recip(1, isl)
                bc0 = emit_bc(r0)
                bc1 = emit_bc(r1)
                o_norm[isl] = (emit_mul(0, isl, bc0), emit_mul(1, isl, bc1))

            def trail_outproj(isl, cts):
                o0, o1 = o_norm[isl]
                for ct in cts:
                    emit_outproj_ct(isl, ct, o0, o1)

            def emit_lead(isl, g, h_first=0):
                emit_scores(h_first, isl, g)
                emit_scores(1 - h_first, isl, g)
                trail_q.append(("pv", lambda isl=isl, g=g: (
                    emit_pv(0, isl, g), emit_pv(1, isl, g))))
                if g == JG - 1 and isl < IT - 1:
                    trail_q.append(("aux", lambda isl=isl: trail_norm(isl)))
                    for cts in ((0, 1), (2, 3), (4, 5)):
                        trail_q.append(
                            ("aux", lambda isl=isl, cts=cts: trail_outproj(isl, cts)))
                lt_state["lead"] += 1
                # taper: near the end of the lead stream, drain the trail
                # deeper so the Act-paced score stalls are filled with PV work
                # and little trail remains after the last scores
                n_lead_total = IT * JG
                taper = max(0, lt_state["lead"] - (n_lead_total - cfg["lag"] + 2))
                pump_trail(lt_state["lead"] - cfg["lag"] + cfg["taper_mult"] * taper)

            # phase 1: slice-pipelined projections, isl0's lead as each key
            # slice lands
            for s in range(IT):
                trickle = cfg["warm_trickle"] if s == 0 else 0
                if s == 0:
                    dummy_mm(cfg["warm0"])
                proj_qk3(s, trickle=trickle)
                if s == 0:
                    emit_q_copies(0)
                for jt in range(4 * s, 4 * s + 4):
                    proj_v(jt)
                if s > 0:
                    emit_q_copies(s)
                for g in range(2 * s, 2 * s + 2):
                    # h1 first: its k tile is consumed in place (no pool
                    # copies), so it runs while k0's shifted copies land
                    emit_lead(0, g, h_first=1)

            # phase 2: remaining slices' lead, trail pumping throughout
            for isl in range(1, IT):
                for g in range(JG):
                    emit_lead(isl, g)
            pump_trail(10 ** 9)

            # ---- tail: isl3 normalize + out-projection + stores ----
            # h0's half of the out-projection starts as soon as o0 is ready
            # (po tiles: 2 from ps_proj + 4 carved from the now-free ps_attn
            # tiles); h1 accumulates into them once o1 lands. Copies alternate
            # DVE/Act per chunk; the earliest stores ride the SWDGE lane.
            isl = IT - 1
            po = [None] * CT

            def mm_out(h, ct, o, stop):
                nc.tensor.matmul(
                    po[ct][:], wo_t[h][:, ct * 128:(ct + 1) * 128], o[:],
                    start=(h == 0), stop=stop,
                )

            # tail pipeline (isl3's PVs already ran in the trail): h0's exp
            # and PV finish first, so h0's normalize + opening out-proj
            # matmuls overlap h1's final exp + PV; h1 closes the accumulation
            # with copy + store chasing each closing matmul.
            r0 = emit_recip(0, isl)
            bc0 = emit_bc(r0)
            o0 = emit_mul(0, isl, bc0, copy_eng=nc.scalar)
            r1 = emit_recip(1, isl)
            CT_ORDER = (2, 3, 4, 5, 0, 1)
            for ct in CT_ORDER[:4]:
                if ct % 2 == 0:
                    big = ps_attn.tile([128, 1024], f32, name="ps_attn")
                    po[ct] = big[:, 0:512]
                else:
                    po[ct] = big[:, 512:1024]
                nc.tensor.matmul(
                    po[ct][:], wo_t[0][:, ct * 128:(ct + 1) * 128], o0[:],
                    start=True, stop=False,
                )
            bc1 = emit_bc(r1)
            o1 = emit_mul(1, isl, bc1, copy_eng=nc.scalar)
            for ct in CT_ORDER[4:]:
                po[ct] = ps_o.tile([128, 512], f32, name="ps_o")
                nc.tensor.matmul(
                    po[ct][:], wo_t[0][:, ct * 128:(ct + 1) * 128], o0[:],
                    start=True, stop=False,
                )
            for i, ct in enumerate(CT_ORDER):
                nc.tensor.matmul(
                    po[ct][:], wo_t[1][:, ct * 128:(ct + 1) * 128], o1[:],
                    start=False, stop=True,
                )
                oc = sb_oc.tile([128, 512], bf16, name="oc")
                if i % 2 == 0:
                    nc.scalar.activation(oc[:], po[ct][:], COPY)
                else:
                    nc.vector.tensor_copy(oc[:], po[ct][:])
                dst = out[ct * 128:(ct + 1) * 128, isl * 512:(isl + 1) * 512]
                if i < cfg["tail_swdge"]:
                    nc.gpsimd.dma_start(dst, oc[:])
                else:
                    nc.sync.dma_start(dst, oc[:])

    _split_sync_waits(nc, mybir)
    return nc


class _Runner:
    """Compile once, run many. Mirrors run_bass_via_pjrt's multi-core path but
    keeps the jitted executable cached across calls."""

    def __init__(self, cfg=None):
        import jax
        import concourse.mybir as mybir
        from concourse import bass2jax
        from jax.sharding import Mesh, PartitionSpec
        from jax.experimental.shard_map import shard_map

        self.jax = jax
        nc = _build_nc(cfg)
        self.nc = nc
        bass2jax.install_neuronx_cc_hook()

        in_names, out_names, out_avals = [], [], []
        for alloc in nc.m.functions[0].allocations:
            if not isinstance(alloc, mybir.MemoryLocationSet):
                continue
            name = alloc.memorylocations[0].name
            if alloc.kind == "ExternalInput":
                if nc.partition_id_tensor is None or name != nc.partition_id_tensor.name:
                    in_names.append(name)
            elif alloc.kind == "ExternalOutput":
                out_names.append(name)
                out_avals.append(
                    jax.core.ShapedArray(tuple(alloc.tensor_shape), mybir.dt.np(alloc.dtype))
                )
        self.in_names = in_names
        self.out_names = out_names
        partition_name = nc.partition_id_tensor.name if nc.partition_id_tensor else None
        all_names = tuple(in_names + out_names + ([partition_name] if partition_name else []))

        def _body(*args):
            operands = list(args)
            if partition_name is not None:
                operands.append(bass2jax.partition_id_tensor())
            outs = bass2jax._bass_exec_p.bind(
                *operands,
                out_avals=tuple(out_avals),
                in_names=all_names,
                out_names=tuple(out_names),
                lowering_input_output_aliases=(),
                sim_require_finite=True,
                sim_require_nnan=True,
                nc=nc,
            )
            return tuple(outs)

        devices = jax.devices()[:N_CORES]
        mesh = Mesh(np.asarray(devices), ("core",))
        n_all = len(in_names) + len(out_names)
        self.sharded = jax.jit(
            shard_map(
                _body,
                mesh=mesh,
                in_specs=(PartitionSpec("core"),) * n_all,
                out_specs=(PartitionSpec("core"),) * len(out_names),
                check_rep=False,
            ),
            keep_unused=True,
        )
        self.out_shapes = [tuple(a.shape) for a in out_avals]
        self.out_dtypes = [a.dtype for a in out_avals]

    def run(self, in_maps):
        concat_in = [
            np.concatenate([np.asarray(in_maps[c][n]) for c in range(N_CORES)], axis=0)
            for n in self.in_names
        ]
        concat_zero = [
            np.zeros((N_CORES * s[0], *s[1:]), d)
            for s, d in zip(self.out_shapes, self.out_dtypes)
        ]
        outs = self.sharded(*concat_in, *concat_zero)
        self.jax.block_until_ready(outs)
        return [
            {
                n: np.asarray(outs[i]).reshape(N_CORES, *self.out_shapes[i])[c]
                for i, n in enumerate(self.out_names)
            }
            for c in range(N_CORES)
        ]


def _get_runner():
    global _RUNNER
    if _RUNNER is None:
        _RUNNER = _Runner()
    return _RUNNER


def _pack_w(w):
    """(768, 192) -> (128, 6*192) partition-major: out[p, ct*192+j] = w[ct*128+p, j]."""
    return np.ascontiguousarray(
        w.reshape(CT, 128, 2 * D).transpose(1, 0, 2).reshape(128, CT * 2 * D)
    )


def _shard_inputs(inputs, W_qkv, W_out):
    import ml_dtypes

    bf16 = ml_dtypes.bfloat16
    in_maps = []
    for core in range(N_CORES):
        b, g = divmod(core, 4)
        cols = slice(g * 2 * D, (g + 1) * 2 * D)
        wo = W_out[cols, :]  # (192, 768)
        wo_packed = np.ascontiguousarray(
            wo.reshape(2, D, C).transpose(1, 0, 2).reshape(D, 2 * C)
        )
        # packed q/k: per c-tile the 384 output dims are [q0|q1|k0|k1]
        q = W_qkv[:, cols].reshape(CT, 128, 2 * D)
        k = W_qkv[:, 768:][:, cols].reshape(CT, 128, 2 * D)
        wqk = np.concatenate([q, k], axis=2)  # (CT, 128, 384)
        wqk = np.ascontiguousarray(
            wqk.transpose(1, 0, 2).reshape(128, CT * 4 * D)
        )
        in_maps.append({
            "x": np.ascontiguousarray(inputs[b]).astype(bf16),
            "wqk": wqk.astype(bf16),
            "wv": _pack_w(W_qkv[:, 1536:][:, cols]).astype(bf16),
            "wo": wo_packed,
        })
    return in_maps


def kernel(inputs, W_qkv, W_out):
    inputs = np.asarray(inputs, dtype=np.float32)
    W_qkv = np.asarray(W_qkv, dtype=np.float32)
    W_out = np.asarray(W_out, dtype=np.float32)
    runner = _get_runner()
    results = runner.run(_shard_inputs(inputs, W_qkv, W_out))
    out = np.zeros((B, C, S), np.float32)
    for core in range(N_CORES):
        out[core // 4] += results[core]["out"].astype(np.float32)
    return out
